# revision 18
# baseline (speedup 1.0000x reference)
"""Trainium2 Bass kernel for nn_AMOE_79843442033161 (ViT-MoE, 4 layers).

Sharding: data-parallel attention (1 image per core, 8 cores) + expert-parallel
MoE (1 expert per core, dense tokens): per layer, AllGather transposed
activations+gates, each core runs its expert over all 2088 tokens,
ReduceScatter (bf16) combines expert outputs back to token owners.

All matmuls fp32r (f32 bytes, full PE rate): requires even moving-N (pad
261->262) and dst start_partition 0. Residual transposed xT [1024, 262] f32
(col 261 junk). Norm weights folded into following matmuls host-side; final
norm weight applied on host.
"""
import os
import sys
import numpy as np

sys.path.insert(0, "/opt/trn_rl_repo")

import concourse.bass as bass
import concourse.mybir as mybir
import concourse.tile as tile
from concourse import bacc, bass_isa
from concourse.bass_utils import run_bass_kernel_spmd
from concourse.alu_op_type import AluOpType
from concourse.masks import make_identity

B, S, D, H, HD, L, E, F = 8, 261, 1024, 16, 64, 4, 8, 1024
SW = 262  # padded token width (fp32r needs even moving-N)
NS = 4
PDIM = 768
EPS = 1e-6
THETA = 10000.0
P = 128
KT = D // P
FT = F // P
PT = PDIM // P
ST = [(0, 128), (128, 128), (256, 5)]
F32 = mybir.dt.float32
F32R = mybir.dt.float32r
BF16 = mybir.dt.bfloat16
AF = mybir.ActivationFunctionType

LAST_RESULT = None


def _host_prep(inputs):
    lp = {k: np.asarray(v) for k, v in inputs["layer_params"].items()}
    out = {}

    def tile_w(w):  # [Din, Dout] f32 -> [128(p), kt, Dout]
        din, dout = w.shape
        kt = din // P
        return np.ascontiguousarray(
            w.astype(np.float32).reshape(kt, P, dout).transpose(1, 0, 2)
        )

    wq = np.empty((L, P, KT, D), np.float32)
    wk = np.empty((L, P, KT, D), np.float32)
    wv = np.empty((L, P, KT, D), np.float32)
    wo = np.empty((L, P, KT, D), np.float32)
    rt = np.empty((L, P, KT, E), np.float32)
    w1 = np.empty((L, E, P, KT, F), np.float32)
    w3 = np.empty((L, E, P, KT, F), np.float32)
    w2 = np.empty((L, E, P, FT, D), np.float32)
    for l in range(L):
        an = lp["attn_norm"][l][:, None].astype(np.float32)
        fn = lp["ffn_norm"][l][:, None].astype(np.float32)
        wq[l] = tile_w(an * lp["wq"][l])
        wk[l] = tile_w(an * lp["wk"][l])
        wv[l] = tile_w(an * lp["wv"][l])
        wo[l] = tile_w(lp["wo"][l])
        rt[l] = tile_w(fn * lp["router"][l])
        for e in range(E):
            w1[l, e] = tile_w(fn * lp["w1"][l, e])
            w3[l, e] = tile_w(fn * lp["w3"][l, e])
            w2[l, e] = tile_w(lp["w2"][l, e])
    out.update(wq=wq, wk=wk, wv=wv, wo=wo, rt=rt, w1=w1, w3=w3, w2=w2)
    out["expsink"] = np.ascontiguousarray(
        np.exp(lp["sink"]).astype(np.float32).reshape(1, L * H)
    )

    w_img = np.asarray(inputs["w_img"]).astype(np.float32)
    out["w_img"] = np.ascontiguousarray(w_img.reshape(PT, P, D).transpose(1, 0, 2))
    cs = np.concatenate(
        [
            np.asarray(inputs["cls_token"]).reshape(1, D),
            np.asarray(inputs["storage"]).reshape(NS, D),
        ],
        0,
    ).astype(np.float32)
    out["clsstorT"] = np.ascontiguousarray(
        cs.T.reshape(KT, P, 1 + NS).transpose(1, 0, 2)
    )

    inv = 1.0 / (THETA ** (np.arange(0, HD, 2, dtype=np.float32) / HD))
    fr = np.outer(np.arange(S, dtype=np.float32), inv)
    cos, sin = np.cos(fr), np.sin(fr)
    CC = np.zeros((P, SW), np.float32)
    SS = np.zeros((P, SW), np.float32)
    for p in range(P):
        d = p % HD
        j = d // 2
        CC[p, :S] = cos[:, j]
        SS[p, :S] = (1.0 if d % 2 == 1 else -1.0) * sin[:, j]
    out["CC"] = CC
    out["SS"] = SS
    perm = np.zeros((P, P), np.float32)
    for p in range(P):
        perm[p ^ 1, p] = 1.0
    out["perm"] = perm
    return out


def build_nc():
    nc = bacc.Bacc("TRN2", target_bir_lowering=False, debug=False, num_devices=B)
    dt = nc.dram_tensor
    t_patches = dt("patchesT", [PT, P, 256], F32R, kind="ExternalInput")
    t_wimg = dt("w_img", [P, PT, D], F32R, kind="ExternalInput")
    t_cs = dt("clsstorT", [P, KT, 1 + NS], F32, kind="ExternalInput")
    t_CC = dt("CC", [P, SW], F32, kind="ExternalInput")
    t_SS = dt("SS", [P, SW], F32, kind="ExternalInput")
    t_perm = dt("perm", [P, P], F32R, kind="ExternalInput")
    t_sink = dt("expsink", [1, L * H], F32, kind="ExternalInput")
    t_wq = dt("wq", [L, P, KT, D], F32R, kind="ExternalInput")
    t_wk = dt("wk", [L, P, KT, D], F32R, kind="ExternalInput")
    t_wv = dt("wv", [L, P, KT, D], F32R, kind="ExternalInput")
    t_wo = dt("wo", [L, P, KT, D], F32R, kind="ExternalInput")
    t_rt = dt("rt", [L, P, KT, E], F32R, kind="ExternalInput")
    t_w1 = dt("w1", [L, P, KT, F], F32R, kind="ExternalInput")
    t_w3 = dt("w3", [L, P, KT, F], F32R, kind="ExternalInput")
    t_w2 = dt("w2", [L, P, FT, D], F32R, kind="ExternalInput")
    t_out = dt("out", [S, D], F32, kind="ExternalOutput")
    t_goffs = dt("goffs", [8, 1], mybir.dt.int32, kind="ExternalInput")

    RG = [list(range(B))]

    with tile.TileContext(nc) as tc:
        import contextlib

        with contextlib.ExitStack() as ctx:
            sb = ctx.enter_context(tc.tile_pool(name="sb", bufs=1))
            sb2 = ctx.enter_context(tc.tile_pool(name="sb2", bufs=2))
            sbe = ctx.enter_context(tc.tile_pool(name="sbe", bufs=6))
            gpool = ctx.enter_context(tc.tile_pool(name="gpool", bufs=2))
            hnb = ctx.enter_context(tc.tile_pool(name="hnb", bufs=18))
            wpool = ctx.enter_context(tc.tile_pool(name="wpool", bufs=2))
            epool = ctx.enter_context(tc.tile_pool(name="epool", bufs=2))
            ps261 = ctx.enter_context(tc.tile_pool(name="ps261", bufs=5, space="PSUM"))
            ps512 = ctx.enter_context(tc.tile_pool(name="ps512", bufs=1, space="PSUM"))
            pssm = ctx.enter_context(tc.tile_pool(name="pssm", bufs=2, space="PSUM"))
            dram = ctx.enter_context(tc.tile_pool(name="dram", bufs=1, space="DRAM"))

            CC_sb = sb.tile([P, SW], F32, tag="CC")
            SS_sb = sb.tile([P, SW], F32, tag="SS")
            perm_sb = sb.tile([P, P], F32R, tag="perm")
            sink_sb = sb.tile([1, L * H], F32, tag="sink")
            ones_sb = sb.tile([P, 1], F32R, tag="ones")
            ones_row = sb.tile([1, P], F32, tag="onesrow")
            ones_rowr = sb.tile([1, P], F32R, tag="onesrowr")
            ident = sb.tile([P, P], F32, tag="ident")
            nc.sync.dma_start(out=CC_sb[:], in_=t_CC[:])
            nc.sync.dma_start(out=SS_sb[:], in_=t_SS[:])
            nc.sync.dma_start(out=perm_sb[:], in_=t_perm[:])
            nc.sync.dma_start(out=sink_sb[:], in_=t_sink[:])
            nc.vector.memset(ones_sb[:].bitcast(F32), 1.0)
            nc.vector.memset(ones_row[:], 1.0)
            nc.vector.memset(ones_rowr[:].bitcast(F32), 1.0)
            eps128 = sb.tile([P, 1], F32, tag="eps")
            nc.vector.memset(eps128[:], EPS)
            make_identity(nc, ident[:])

            xT = [sb.tile([P, SW], F32, tag=f"xT{m}", name=f"xT{m}") for m in range(KT)]
            normT = [sb.tile([P, SW], F32R, tag=f"nT{m}", name=f"nT{m}") for m in range(KT)]
            qTr = [sb.tile([P, SW], F32R, tag=f"qTr{m}", name=f"qTr{m}") for m in range(KT)]
            kTr = [sb.tile([P, SW], F32R, tag=f"kTr{m}", name=f"kTr{m}") for m in range(KT)]
            attnT = [sb.tile([P, SW], F32R, tag=f"aT{m}", name=f"aT{m}") for m in range(KT)]
            vext = [sb.tile([P, H * (HD + 1)], F32R, tag=f"vx{i}", name=f"vx{i}") for i in range(3)]
            zbuf = sb.tile([P, SW], F32, tag="zbuf")
            gall = sb.tile([1, B * SW], F32R, tag="gall")

            agin = dram.tile([D + E, S], F32R)
            agouts = [
                dram.tile([B * (D + E), S], F32R, addr_space="Shared",
                          tag=f"agout{i}", name=f"agout{i}")
                for i in range(L)
            ]
            rsin = dram.tile([B * D, S], BF16)
            rsout = dram.tile([D, S], BF16)

            # ---- embed ----
            for m in range(KT):
                ps = ps512.tile([P, 512], F32, tag="ps512")
                for k in range(PT):
                    wi = wpool.tile([P, P], F32R, tag="wmat")
                    nc.sync.dma_start(out=wi[:], in_=t_wimg[:, k, m * P : (m + 1) * P])
                    pk = hnb.tile([P, 256], F32R, tag="hnb")
                    nc.sync.dma_start(out=pk[:], in_=t_patches[k])
                    nc.tensor.matmul(
                        out=ps[:, :256],
                        lhsT=wi[:],
                        rhs=pk[:],
                        start=(k == 0),
                        stop=(k == PT - 1),
                    )
                nc.vector.tensor_copy(
                    out=xT[m][:, 1 + NS : 1 + NS + 256], in_=ps[:, :256]
                )
                csm = sb2.tile([P, 1 + NS], F32, tag="csm")
                nc.sync.dma_start(out=csm[:], in_=t_cs[:, m, :])
                nc.vector.tensor_copy(out=xT[m][:, : 1 + NS], in_=csm[:])
                nc.vector.memset(xT[m][:, S:SW], 0.0)

            def rmsnorm(dst):
                ssq = pssm.tile([P, SW], F32, tag="pssm")
                for m in range(KT):
                    sqm = sb2.tile([P, SW], F32R, tag="sq")
                    nc.vector.tensor_tensor(
                        out=sqm[:], in0=xT[m][:], in1=xT[m][:], op=AluOpType.mult
                    )
                    nc.tensor.matmul(
                        out=ssq[0:1, :],
                        lhsT=ones_sb[:],
                        rhs=sqm[:],
                        start=(m == 0),
                        stop=(m == KT - 1),
                    )
                lnv = sb2.tile([1, SW], F32, tag="lnv")
                nc.scalar.activation(
                    out=lnv[:], in_=ssq[0:1, :], func=AF.Ln,
                    bias=eps128[0:1, :], scale=1.0 / D,
                )
                rstd = sb2.tile([1, SW], F32, tag="rstd")
                nc.scalar.activation(out=rstd[:], in_=lnv[:], func=AF.Exp, scale=-0.5)
                zps = pssm.tile([P, SW], F32, tag="pssm")
                nc.tensor.matmul(
                    out=zps[:], lhsT=ones_row[:], rhs=rstd[:], start=True, stop=True
                )
                nc.scalar.activation(out=zbuf[:], in_=zps[:], func=AF.Copy)
                for m in range(KT):
                    nc.vector.tensor_tensor(
                        out=dst[m][:], in0=xT[m][:], in1=zbuf[:], op=AluOpType.mult
                    )

            def load_half(tdram, l, mh, pool):
                w = pool.tile([P, KT * 512], F32R, tag="wmat")
                nc.sync.dma_start(
                    out=w[:].rearrange("p (k c) -> p k c", k=KT),
                    in_=tdram[l][:, :, mh * 512 : (mh + 1) * 512],
                )
                return w

            def load_quarter(tdram, l, mq, pool):
                w = pool.tile([P, KT * 256], F32R, tag="wmat")
                nc.sync.dma_start(
                    out=w[:].rearrange("p (k c) -> p k c", k=KT),
                    in_=tdram[l][:, :, mq * 256 : (mq + 1) * 256],
                )
                return w

            for l in range(L):
                # ======== attention ========
                rmsnorm(normT)

                def proj_rope(tdram, dstT):
                    for mh in range(4):
                        w_sb = load_quarter(tdram, l, mh, wpool)
                        for mi in range(2):
                            m = mh * 2 + mi
                            ps = ps261.tile([P, SW], F32, tag="ps261")
                            for k in range(KT):
                                nc.tensor.matmul(
                                    out=ps[:],
                                    lhsT=w_sb[
                                        :, k * 256 + mi * P : k * 256 + (mi + 1) * P
                                    ],
                                    rhs=normT[k][:],
                                    start=(k == 0),
                                    stop=(k == KT - 1),
                                )
                            raw = sb2.tile([P, SW], F32R, tag="rawq")
                            nc.scalar.activation(out=raw[:], in_=ps[:], func=AF.Copy)
                            sw = ps261.tile([P, SW], F32, tag="ps261")
                            nc.tensor.matmul(
                                out=sw[:], lhsT=perm_sb[:], rhs=raw[:],
                                start=True, stop=True,
                            )
                            t1 = sb2.tile([P, SW], F32, tag="t1")
                            nc.vector.tensor_tensor(
                                out=t1[:], in0=raw[:], in1=CC_sb[:], op=AluOpType.mult
                            )
                            t2 = sb2.tile([P, SW], F32, tag="t2")
                            nc.vector.tensor_tensor(
                                out=t2[:], in0=sw[:], in1=SS_sb[:], op=AluOpType.mult
                            )
                            nc.vector.tensor_tensor(
                                out=dstT[m][:], in0=t1[:], in1=t2[:], op=AluOpType.add
                            )

                proj_rope(t_wq, qTr)
                proj_rope(t_wk, kTr)

                for mh in range(4):
                    wv_sb = load_quarter(t_wv, l, mh, wpool)
                    for si, (s0, sl) in enumerate(ST):
                        vx = vext[si]
                        if mh == 0:
                            nc.vector.memset(
                                vx[:].bitcast(F32).rearrange(
                                    "p (h c) -> p h c", c=HD + 1
                                )[:, :, HD:],
                                1.0,
                            )
                        ps = ps512.tile([P, 512], F32, tag="ps512")
                        for k in range(KT):
                            nc.tensor.matmul(
                                out=ps[:sl, :256],
                                lhsT=normT[k][:, s0 : s0 + sl],
                                rhs=wv_sb[:, k * 256 : (k + 1) * 256],
                                start=(k == 0),
                                stop=(k == KT - 1),
                            )
                        nc.vector.tensor_copy(
                            out=vx[:sl].rearrange("p (h c) -> p h c", c=HD + 1)[
                                :, mh * 4 : (mh + 1) * 4, :HD
                            ],
                            in_=ps[:sl, :256].rearrange("p (h c) -> p h c", c=HD),
                        )

                # per head: scores->exp->attn+denom->normalize
                for h in range(H):
                    j, hh = divmod(h, 2)
                    tq = qTr[j][hh * HD : (hh + 1) * HD, :]
                    tk = kTr[j][hh * HD : (hh + 1) * HD, :]
                    expts = []
                    dps = pssm.tile([P, SW], F32, tag="pssm")
                    aps = ps261.tile([P, SW], F32, tag="ps261")
                    for si, (s0, sl) in enumerate(ST):
                        sps = ps261.tile([P, SW], F32, tag="ps261")
                        nc.tensor.matmul(
                            out=sps[:sl, :],
                            lhsT=tk[:, s0 : s0 + sl],
                            rhs=tq,
                            start=True,
                            stop=True,
                        )
                        et = sbe.tile([P, SW], F32R, tag="expT")
                        nc.scalar.activation(
                            out=et[:sl, :], in_=sps[:sl, :], func=AF.Exp,
                            scale=float(1.0 / np.sqrt(HD)),
                        )
                        expts.append(et)
                    for si, (s0, sl) in enumerate(ST):
                        nc.tensor.matmul(
                            out=dps[0:1, :],
                            lhsT=vext[si][
                                :sl, h * (HD + 1) + HD : (h + 1) * (HD + 1)
                            ],
                            rhs=expts[si][:sl, :],
                            start=(si == 0),
                            stop=(si == 2),
                        )
                        nc.tensor.matmul(
                            out=aps[0:HD, :],
                            lhsT=vext[si][:sl, h * (HD + 1) : h * (HD + 1) + HD],
                            rhs=expts[si][:sl, :],
                            start=(si == 0),
                            stop=(si == 2),
                        )
                    lnd = sb2.tile([1, SW], F32, tag="lnd")
                    nc.scalar.activation(
                        out=lnd[:], in_=dps[0:1, :], func=AF.Ln,
                        bias=sink_sb[0:1, l * H + h : l * H + h + 1],
                    )
                    z = sb2.tile([1, SW], F32, tag="z")
                    nc.scalar.activation(out=z[:], in_=lnd[:], func=AF.Exp, scale=-1.0)
                    zdp = pssm.tile([P, SW], F32, tag="pssm")
                    nc.tensor.matmul(
                        out=zdp[0:HD, :], lhsT=ones_row[0:1, 0:HD], rhs=z[:],
                        start=True, stop=True,
                    )
                    zd = sb2.tile([P, SW], F32, tag="zd")
                    nc.scalar.activation(out=zd[0:HD, :], in_=zdp[0:HD, :], func=AF.Copy)
                    if hh == 0:
                        nc.vector.tensor_tensor(
                            out=attnT[j][0:HD, :], in0=aps[0:HD, :],
                            in1=zd[0:HD, :], op=AluOpType.mult,
                        )
                    else:
                        tmp = sb2.tile([P, SW], F32R, tag="atmp")
                        nc.vector.tensor_tensor(
                            out=tmp[0:HD, :], in0=aps[0:HD, :],
                            in1=zd[0:HD, :], op=AluOpType.mult,
                        )
                        nc.sync.dma_start(out=attnT[j][HD:P, :], in_=tmp[0:HD, :])

                # wo + residual
                for mh in range(4):
                    wo_sb = load_quarter(t_wo, l, mh, wpool)
                    for mi in range(2):
                        m = mh * 2 + mi
                        ps = ps261.tile([P, SW], F32, tag="ps261")
                        for k in range(KT):
                            nc.tensor.matmul(
                                out=ps[:],
                                lhsT=wo_sb[
                                    :, k * 256 + mi * P : k * 256 + (mi + 1) * P
                                ],
                                rhs=attnT[k][:],
                                start=(k == 0),
                                stop=(k == KT - 1),
                            )
                        nc.vector.tensor_tensor(
                            out=xT[m][:], in0=xT[m][:], in1=ps[:], op=AluOpType.add
                        )

                # ======== MoE ========
                rmsnorm(normT)
                rt_sb = sb2.tile([P, KT * E], F32R, tag="rt")
                nc.sync.dma_start(
                    out=rt_sb[:].rearrange("p (k c) -> p k c", k=KT), in_=t_rt[l]
                )
                gt = sb2.tile([E, SW], F32R, tag="gt")
                for si, (s0, sl) in enumerate(ST):
                    lg_ps = pssm.tile([P, SW], F32, tag="pssm")
                    for k in range(KT):
                        nc.tensor.matmul(
                            out=lg_ps[:sl, :E],
                            lhsT=normT[k][:, s0 : s0 + sl],
                            rhs=rt_sb[:, 0 : KT * E].rearrange(
                                "p (k c) -> p k c", k=KT
                            )[:, k, :],
                            start=(k == 0),
                            stop=(k == KT - 1),
                        )
                    lgn = sb2.tile([P, E], F32, tag="lgn")
                    nc.vector.tensor_copy(out=lgn[:sl, :], in_=lg_ps[:sl, :E])
                    m1 = sb2.tile([P, 1], F32, tag="m1")
                    nc.vector.tensor_reduce(
                        out=m1[:sl, :], in_=lgn[:sl, :],
                        axis=mybir.AxisListType.X, op=AluOpType.max,
                    )
                    mask1 = sb2.tile([P, E], F32, tag="mask1")
                    nc.vector.tensor_scalar(
                        out=mask1[:sl, :], in0=lgn[:sl, :], scalar1=m1[:sl, :],
                        scalar2=None, op0=AluOpType.is_equal,
                    )
                    l2t = sb2.tile([P, E], F32, tag="l2t")
                    nc.vector.tensor_scalar(
                        out=l2t[:sl, :], in0=mask1[:sl, :], scalar1=-1e30,
                        scalar2=None, op0=AluOpType.mult,
                    )
                    nc.vector.tensor_tensor(
                        out=l2t[:sl, :], in0=l2t[:sl, :], in1=lgn[:sl, :],
                        op=AluOpType.add,
                    )
                    m2 = sb2.tile([P, 1], F32, tag="m2")
                    nc.vector.tensor_reduce(
                        out=m2[:sl, :], in_=l2t[:sl, :],
                        axis=mybir.AxisListType.X, op=AluOpType.max,
                    )
                    mask2 = sb2.tile([P, E], F32, tag="mask2")
                    nc.vector.tensor_scalar(
                        out=mask2[:sl, :], in0=l2t[:sl, :], scalar1=m2[:sl, :],
                        scalar2=None, op0=AluOpType.is_equal,
                    )
                    diff = sb2.tile([P, 1], F32, tag="diff")
                    nc.vector.tensor_tensor(
                        out=diff[:sl, :], in0=m2[:sl, :], in1=m1[:sl, :],
                        op=AluOpType.subtract,
                    )
                    gsec = sb2.tile([P, 1], F32, tag="gsec")
                    nc.scalar.activation(out=gsec[:sl, :], in_=diff[:sl, :], func=AF.Sigmoid)
                    gpri = sb2.tile([P, 1], F32, tag="gpri")
                    nc.vector.tensor_scalar(
                        out=gpri[:sl, :], in0=gsec[:sl, :], scalar1=-1.0,
                        scalar2=1.0, op0=AluOpType.mult, op1=AluOpType.add,
                    )
                    gtn = sb2.tile([P, E], F32, tag="gtn")
                    nc.vector.tensor_scalar(
                        out=gtn[:sl, :], in0=mask1[:sl, :], scalar1=gpri[:sl, :],
                        scalar2=None, op0=AluOpType.mult,
                    )
                    gtn2 = sb2.tile([P, E], F32, tag="gtn2")
                    nc.vector.tensor_scalar(
                        out=gtn2[:sl, :], in0=mask2[:sl, :], scalar1=gsec[:sl, :],
                        scalar2=None, op0=AluOpType.mult,
                    )
                    nc.vector.tensor_tensor(
                        out=gtn[:sl, :], in0=gtn[:sl, :], in1=gtn2[:sl, :],
                        op=AluOpType.add,
                    )
                    # transpose [sl, 8] -> [8, sl] into gt columns
                    gtp = pssm.tile([P, SW], F32, tag="pssm")
                    nc.tensor.transpose(
                        out=gtp[:E, :sl], in_=gtn[:sl, :E],
                        identity=ident[:sl, :sl],
                    )
                    nc.vector.tensor_copy(
                        out=gt[:, s0 : s0 + sl], in_=gtp[:E, :sl]
                    )

                # AllGather [normT; gateT]
                for k in range(KT):
                    nc.sync.dma_start(
                        out=agin[k * P : (k + 1) * P, :], in_=normT[k][:, 0:S]
                    )
                nc.sync.dma_start(out=agin[D : D + E, :], in_=gt[:, 0:S])
                agout = agouts[l]
                nc.gpsimd.collective_compute(
                    "AllGather",
                    mybir.AluOpType.bypass,
                    ins=[agin[:].opt()],
                    outs=[agout[:].opt()],
                    replica_groups=RG,
                )

                # my expert's gate rows via indirect row gather + flatten
                goffs_sb = sb2.tile([8, 1], mybir.dt.int32, tag="goffs")
                nc.sync.dma_start(out=goffs_sb[:], in_=t_goffs[:])
                gpart = sb2.tile([8, SW], F32R, tag="gpart")
                nc.gpsimd.indirect_dma_start(
                    out=gpart[:, 0:S],
                    out_offset=None,
                    in_=agout[:],
                    in_offset=bass.IndirectOffsetOnAxis(ap=goffs_sb[:, :1], axis=0),
                )
                nc.sync.dma_start(
                    out=gall[0:1, :].rearrange("p (b s) -> p b s", b=B),
                    in_=gpart[:],
                )

                for bp in range(4):  # block pairs
                    hnbt = []
                    for bi in range(2):
                        b = bp * 2 + bi
                        row = []
                        for k in range(KT):
                            hk = hnb.tile([P, SW], F32R, tag="hnb",
                                          name=f"hn{l}_{b}_{k}")
                            nc.sync.dma_start(
                                out=hk[:, 0:S],
                                in_=agout[
                                    b * (D + E) + k * P : b * (D + E) + (k + 1) * P,
                                    :,
                                ],
                            )
                            row.append(hk)
                        hnbt.append(row)
                    gated = [
                        [
                            gpool.tile([P, SW], F32R, tag=f"gated{bi}_{m}",
                                       name=f"gated{l}_{bp}_{bi}_{m}")
                            for m in range(FT)
                        ]
                        for bi in range(2)
                    ]
                    for q in range(4):
                        w1_sb = load_quarter(t_w1, l, q, epool)
                        w3_sb = load_quarter(t_w3, l, q, epool)
                        for bi in range(2):
                            for mi in range(2):
                                m = q * 2 + mi
                                ps1 = ps261.tile([P, SW], F32, tag="ps261")
                                for k in range(KT):
                                    nc.tensor.matmul(
                                        out=ps1[:],
                                        lhsT=w1_sb[
                                            :, k * 256 + mi * P : k * 256 + (mi + 1) * P
                                        ],
                                        rhs=hnbt[bi][k][:],
                                        start=(k == 0),
                                        stop=(k == KT - 1),
                                    )
                                sil = sb2.tile([P, SW], F32, tag="sil")
                                nc.scalar.activation(
                                    out=sil[:], in_=ps1[:], func=AF.Silu
                                )
                                ps3 = ps261.tile([P, SW], F32, tag="ps261")
                                for k in range(KT):
                                    nc.tensor.matmul(
                                        out=ps3[:],
                                        lhsT=w3_sb[
                                            :, k * 256 + mi * P : k * 256 + (mi + 1) * P
                                        ],
                                        rhs=hnbt[bi][k][:],
                                        start=(k == 0),
                                        stop=(k == KT - 1),
                                    )
                                nc.vector.tensor_tensor(
                                    out=gated[bi][m][:], in0=ps3[:], in1=sil[:],
                                    op=AluOpType.mult,
                                )
                    zgs = []
                    for bi in range(2):
                        b = bp * 2 + bi
                        zgp = pssm.tile([P, SW], F32, tag="pssm")
                        nc.tensor.matmul(
                            out=zgp[:],
                            lhsT=ones_rowr[:],
                            rhs=gall[0:1, b * SW : (b + 1) * SW],
                            start=True, stop=True,
                        )
                        zg = sb2.tile([P, SW], F32, tag="zg")
                        nc.scalar.activation(out=zg[:], in_=zgp[:], func=AF.Copy)
                        zgs.append(zg)
                    for grp in range(4):
                        w2_sb = load_quarter(t_w2, l, grp, epool)
                        for bi in range(2):
                            b = bp * 2 + bi
                            pss = [
                                ps261.tile([P, SW], F32, tag="ps261",
                                           name=f"yps{l}_{b}_{grp}_{i}")
                                for i in range(2)
                            ]
                            for mi in range(2):
                                for k in range(FT):
                                    nc.tensor.matmul(
                                        out=pss[mi][:],
                                        lhsT=w2_sb[
                                            :, k * 256 + mi * P : k * 256 + (mi + 1) * P
                                        ],
                                        rhs=gated[bi][k][:],
                                        start=(k == 0),
                                        stop=(k == FT - 1),
                                    )
                            for mi in range(2):
                                m = grp * 2 + mi
                                yb = sb2.tile([P, SW], BF16, tag="yb")
                                nc.vector.tensor_tensor(
                                    out=yb[:], in0=pss[mi][:], in1=zgs[bi][:],
                                    op=AluOpType.mult,
                                )
                                nc.sync.dma_start(
                                    out=rsin[b * D + m * P : b * D + (m + 1) * P, :],
                                    in_=yb[:, 0:S],
                                )

                nc.gpsimd.collective_compute(
                    "ReduceScatter",
                    mybir.AluOpType.add,
                    ins=[rsin[:].opt()],
                    outs=[rsout[:].opt()],
                    replica_groups=RG,
                )
                for m in range(KT):
                    yk = sb2.tile([P, SW], BF16, tag="yk")
                    nc.sync.dma_start(
                        out=yk[:, 0:S], in_=rsout[m * P : (m + 1) * P, :]
                    )
                    nc.vector.tensor_tensor(
                        out=xT[m][:, 0:S], in0=xT[m][:, 0:S], in1=yk[:, 0:S],
                        op=AluOpType.add,
                    )

            # ---- final norm + transpose out ----
            out_sb = [sb.tile([P, D], F32, tag=f"out{i}", name=f"osb{i}") for i in range(3)]
            ssqf = pssm.tile([P, SW], F32, tag="pssm")
            for m in range(KT):
                sqf = sb2.tile([P, SW], F32R, tag="sq")
                nc.vector.tensor_tensor(
                    out=sqf[:], in0=xT[m][:], in1=xT[m][:], op=AluOpType.mult
                )
                nc.tensor.matmul(
                    out=ssqf[0:1, :], lhsT=ones_sb[:], rhs=sqf[:],
                    start=(m == 0), stop=(m == KT - 1),
                )
            lnf = sb2.tile([1, SW], F32, tag="lnv")
            nc.scalar.activation(
                out=lnf[:], in_=ssqf[0:1, :], func=AF.Ln, bias=eps128[0:1, :],
                scale=1.0 / D,
            )
            rstdf = sb2.tile([1, SW], F32, tag="rstdf")
            nc.scalar.activation(out=rstdf[:], in_=lnf[:], func=AF.Exp, scale=-0.5)
            zpsf = pssm.tile([P, SW], F32, tag="pssm")
            nc.tensor.matmul(
                out=zpsf[:], lhsT=ones_row[:], rhs=rstdf[:], start=True, stop=True
            )
            nc.scalar.activation(out=zbuf[:], in_=zpsf[:], func=AF.Copy)
            for m in range(KT):
                xnf = sb2.tile([P, SW], F32, tag="xnf")
                nc.vector.tensor_tensor(
                    out=xnf[:], in0=xT[m][:], in1=zbuf[:], op=AluOpType.mult
                )
                for si, (s0, sl) in enumerate(ST):
                    tp = ps261.tile([P, SW], F32, tag="ps261")
                    nc.tensor.transpose(
                        out=tp[:sl, :P], in_=xnf[:, s0 : s0 + sl], identity=ident[:]
                    )
                    nc.vector.tensor_copy(
                        out=out_sb[si][:sl, m * P : (m + 1) * P], in_=tp[:sl, :P]
                    )
            for si, (s0, sl) in enumerate(ST):
                nc.sync.dma_start(out=t_out[s0 : s0 + sl, :], in_=out_sb[si][:sl, :])

    nc.compile()
    return nc


_NC_CACHE = None


def kernel(**inputs):
    global LAST_RESULT, _NC_CACHE
    prep = _host_prep(inputs)
    in_maps = []
    for b in range(B):
        patches = np.asarray(inputs["patches"][b]).astype(np.float32)
        pT = np.ascontiguousarray(patches.T.reshape(PT, P, 256))
        in_maps.append(
            {
                "patchesT": pT,
                "w_img": prep["w_img"],
                "clsstorT": prep["clsstorT"],
                "CC": prep["CC"],
                "SS": prep["SS"],
                "perm": prep["perm"],
                "expsink": prep["expsink"],
                "wq": prep["wq"],
                "wk": prep["wk"],
                "wv": prep["wv"],
                "wo": prep["wo"],
                "rt": prep["rt"],
                "w1": np.ascontiguousarray(prep["w1"][:, b]),
                "w3": np.ascontiguousarray(prep["w3"][:, b]),
                "w2": np.ascontiguousarray(prep["w2"][:, b]),
                "goffs": np.array(
                    [[bb * (D + E) + D + b] for bb in range(B)], np.int32
                ),
            }
        )
    if _NC_CACHE is None:
        _NC_CACHE = build_nc()
    nc = _NC_CACHE
    trace = bool(int(os.environ.get("KERNEL_TRACE", "0")))
    if trace:
        try:
            import axon_profile_hook

            axon_profile_hook.install()
        except Exception:
            pass
    res = run_bass_kernel_spmd(nc, in_maps, core_ids=list(range(B)), trace=trace)
    LAST_RESULT = res
    out = np.stack([res.results[b]["out"] for b in range(B)], 0)
    final_w = np.asarray(inputs["final_norm_w"]).astype(np.float32)
    return out * final_w[None, None, :]


# revision 20
# speedup vs baseline: 1.1375x; 1.1375x over previous
"""Trainium2 Bass kernel for nn_AMOE_79843442033161 (ViT-MoE, 4 layers).

Sharding: data-parallel attention (1 image per core, 8 cores) + expert-parallel
MoE (1 expert per core, dense tokens): per layer, AllGather transposed
activations+gates, each core runs its expert over all 2088 tokens,
ReduceScatter (bf16) combines expert outputs back to token owners.

All matmuls fp32r (f32 bytes, full PE rate): requires even moving-N (pad
261->262) and dst start_partition 0. Residual transposed xT [1024, 262] f32
(col 261 junk). Norm weights folded into following matmuls host-side; final
norm weight applied on host.
"""
import os
import sys
import numpy as np

sys.path.insert(0, "/opt/trn_rl_repo")

import concourse.bass as bass
import concourse.mybir as mybir
import concourse.tile as tile
from concourse import bacc, bass_isa
from concourse.bass_utils import run_bass_kernel_spmd
from concourse.alu_op_type import AluOpType
from concourse.masks import make_identity

B, S, D, H, HD, L, E, F = 8, 261, 1024, 16, 64, 4, 8, 1024
SW = 262  # padded token width (fp32r needs even moving-N)
NS = 4
PDIM = 768
EPS = 1e-6
THETA = 10000.0
P = 128
KT = D // P
FT = F // P
PT = PDIM // P
ST = [(0, 128), (128, 128), (256, 5)]
F32 = mybir.dt.float32
F32R = mybir.dt.float32r
BF16 = mybir.dt.bfloat16
AF = mybir.ActivationFunctionType

LAST_RESULT = None


def _host_prep(inputs):
    lp = {k: np.asarray(v) for k, v in inputs["layer_params"].items()}
    out = {}

    def tile_w(w):  # [Din, Dout] f32 -> [128(p), kt, Dout]
        din, dout = w.shape
        kt = din // P
        return np.ascontiguousarray(
            w.astype(np.float32).reshape(kt, P, dout).transpose(1, 0, 2)
        )

    def tile_w4(w):  # [Din, Dout] -> [4(q), 128(p), kt, Dout//4] contiguous
        t = tile_w(w)  # [P, kt, Dout]
        p_, kt_, dout = t.shape
        return np.ascontiguousarray(
            t.reshape(p_, kt_, 4, dout // 4).transpose(2, 0, 1, 3)
        )

    wq = np.empty((L, 4, P, KT, D // 4), np.float32)
    wk = np.empty((L, 4, P, KT, D // 4), np.float32)
    wv = np.empty((L, 4, P, KT, D // 4), np.float32)
    wo = np.empty((L, 4, P, KT, D // 4), np.float32)
    rt = np.empty((L, P, KT, E), np.float32)
    w1 = np.empty((L, E, 4, P, KT, F // 4), np.float32)
    w3 = np.empty((L, E, 4, P, KT, F // 4), np.float32)
    w2 = np.empty((L, E, 4, P, FT, D // 4), np.float32)
    for l in range(L):
        an = lp["attn_norm"][l][:, None].astype(np.float32)
        fn = lp["ffn_norm"][l][:, None].astype(np.float32)
        wq[l] = tile_w4(an * lp["wq"][l])
        wk[l] = tile_w4(an * lp["wk"][l])
        wv[l] = tile_w4(an * lp["wv"][l])
        wo[l] = tile_w4(lp["wo"][l])
        rt[l] = tile_w(fn * lp["router"][l])
        for e in range(E):
            w1[l, e] = tile_w4(fn * lp["w1"][l, e])
            w3[l, e] = tile_w4(fn * lp["w3"][l, e])
            w2[l, e] = tile_w4(lp["w2"][l, e])
    out.update(wq=wq, wk=wk, wv=wv, wo=wo, rt=rt, w1=w1, w3=w3, w2=w2)
    out["expsink"] = np.ascontiguousarray(
        np.exp(lp["sink"]).astype(np.float32).reshape(1, L * H)
    )

    w_img = np.asarray(inputs["w_img"]).astype(np.float32)
    out["w_img"] = np.ascontiguousarray(w_img.reshape(PT, P, D).transpose(1, 0, 2))
    cs = np.concatenate(
        [
            np.asarray(inputs["cls_token"]).reshape(1, D),
            np.asarray(inputs["storage"]).reshape(NS, D),
        ],
        0,
    ).astype(np.float32)
    out["clsstorT"] = np.ascontiguousarray(
        cs.T.reshape(KT, P, 1 + NS).transpose(1, 0, 2)
    )

    inv = 1.0 / (THETA ** (np.arange(0, HD, 2, dtype=np.float32) / HD))
    fr = np.outer(np.arange(S, dtype=np.float32), inv)
    cos, sin = np.cos(fr), np.sin(fr)
    CC = np.zeros((P, SW), np.float32)
    SS = np.zeros((P, SW), np.float32)
    for p in range(P):
        d = p % HD
        j = d // 2
        CC[p, :S] = cos[:, j]
        SS[p, :S] = (1.0 if d % 2 == 1 else -1.0) * sin[:, j]
    out["CC"] = CC
    out["SS"] = SS
    perm = np.zeros((P, P), np.float32)
    for p in range(P):
        perm[p ^ 1, p] = 1.0
    out["perm"] = perm
    return out


def build_nc():
    nc = bacc.Bacc("TRN2", target_bir_lowering=False, debug=False, num_devices=B)
    dt = nc.dram_tensor
    t_patches = dt("patchesT", [PT, P, 256], F32R, kind="ExternalInput")
    t_wimg = dt("w_img", [P, PT, D], F32R, kind="ExternalInput")
    t_cs = dt("clsstorT", [P, KT, 1 + NS], F32, kind="ExternalInput")
    t_CC = dt("CC", [P, SW], F32, kind="ExternalInput")
    t_SS = dt("SS", [P, SW], F32, kind="ExternalInput")
    t_perm = dt("perm", [P, P], F32R, kind="ExternalInput")
    t_sink = dt("expsink", [1, L * H], F32, kind="ExternalInput")
    t_wq = dt("wq", [L, 4, P, KT, D // 4], F32R, kind="ExternalInput")
    t_wk = dt("wk", [L, 4, P, KT, D // 4], F32R, kind="ExternalInput")
    t_wv = dt("wv", [L, 4, P, KT, D // 4], F32R, kind="ExternalInput")
    t_wo = dt("wo", [L, 4, P, KT, D // 4], F32R, kind="ExternalInput")
    t_rt = dt("rt", [L, P, KT, E], F32R, kind="ExternalInput")
    t_w1 = dt("w1", [L, 4, P, KT, F // 4], F32R, kind="ExternalInput")
    t_w3 = dt("w3", [L, 4, P, KT, F // 4], F32R, kind="ExternalInput")
    t_w2 = dt("w2", [L, 4, P, FT, D // 4], F32R, kind="ExternalInput")
    t_out = dt("out", [S, D], F32, kind="ExternalOutput")
    t_goffs = dt("goffs", [8, 1], mybir.dt.int32, kind="ExternalInput")

    RG = [list(range(B))]

    with tile.TileContext(nc) as tc:
        import contextlib

        with contextlib.ExitStack() as ctx:
            sb = ctx.enter_context(tc.tile_pool(name="sb", bufs=1))
            sb2 = ctx.enter_context(tc.tile_pool(name="sb2", bufs=2))
            sbe = ctx.enter_context(tc.tile_pool(name="sbe", bufs=6))
            gpool = ctx.enter_context(tc.tile_pool(name="gpool", bufs=2))
            hnb = ctx.enter_context(tc.tile_pool(name="hnb", bufs=18))
            wpool = ctx.enter_context(tc.tile_pool(name="wpool", bufs=2))
            epool = ctx.enter_context(tc.tile_pool(name="epool", bufs=2))
            ps261 = ctx.enter_context(tc.tile_pool(name="ps261", bufs=4, space="PSUM"))
            ps512 = ctx.enter_context(tc.tile_pool(name="ps512", bufs=2, space="PSUM"))
            pssm = ctx.enter_context(tc.tile_pool(name="pssm", bufs=2, space="PSUM"))
            dram = ctx.enter_context(tc.tile_pool(name="dram", bufs=1, space="DRAM"))

            CC_sb = sb.tile([P, SW], F32, tag="CC")
            SS_sb = sb.tile([P, SW], F32, tag="SS")
            perm_sb = sb.tile([P, P], F32R, tag="perm")
            sink_sb = sb.tile([1, L * H], F32, tag="sink")
            ones_sb = sb.tile([P, 1], F32R, tag="ones")
            ones_row = sb.tile([1, P], F32, tag="onesrow")
            ones_rowr = sb.tile([1, P], F32R, tag="onesrowr")
            ident = sb.tile([P, P], F32, tag="ident")
            nc.sync.dma_start(out=CC_sb[:], in_=t_CC[:])
            nc.sync.dma_start(out=SS_sb[:], in_=t_SS[:])
            nc.sync.dma_start(out=perm_sb[:], in_=t_perm[:])
            nc.sync.dma_start(out=sink_sb[:], in_=t_sink[:])
            nc.vector.memset(ones_sb[:].bitcast(F32), 1.0)
            nc.vector.memset(ones_row[:], 1.0)
            nc.vector.memset(ones_rowr[:].bitcast(F32), 1.0)
            eps128 = sb.tile([P, 1], F32, tag="eps")
            nc.vector.memset(eps128[:], EPS)
            make_identity(nc, ident[:])

            xT = [sb.tile([P, SW], F32, tag=f"xT{m}", name=f"xT{m}") for m in range(KT)]
            normT = [sb.tile([P, SW], F32R, tag=f"nT{m}", name=f"nT{m}") for m in range(KT)]
            qTr = [sb.tile([P, SW], F32R, tag=f"qTr{m}", name=f"qTr{m}") for m in range(KT)]
            kTr = [sb.tile([P, SW], F32R, tag=f"kTr{m}", name=f"kTr{m}") for m in range(KT)]
            attnT = [sb.tile([P, SW], F32R, tag=f"aT{m}", name=f"aT{m}") for m in range(KT)]
            vext = [sb.tile([P, H * (HD + 1)], F32R, tag=f"vx{i}", name=f"vx{i}") for i in range(3)]
            zbuf = sb.tile([P, SW], F32, tag="zbuf")
            gall = sb.tile([1, B * SW], F32R, tag="gall")

            agin = dram.tile([D + E, S], F32R)
            agouts = [
                dram.tile([B * (D + E), S], F32R, addr_space="Shared",
                          tag=f"agout{i}", name=f"agout{i}")
                for i in range(L)
            ]
            rsin = dram.tile([B * D, S], BF16)
            rsout = dram.tile([D, S], BF16)

            # ---- embed ----
            for m in range(KT):
                ps = ps512.tile([P, 512], F32, tag="ps512")
                for k in range(PT):
                    wi = wpool.tile([P, P], F32R, tag="wmat")
                    nc.sync.dma_start(out=wi[:], in_=t_wimg[:, k, m * P : (m + 1) * P])
                    pk = hnb.tile([P, 256], F32R, tag="hnb")
                    nc.sync.dma_start(out=pk[:], in_=t_patches[k])
                    nc.tensor.matmul(
                        out=ps[:, :256],
                        lhsT=wi[:],
                        rhs=pk[:],
                        start=(k == 0),
                        stop=(k == PT - 1),
                    )
                nc.vector.tensor_copy(
                    out=xT[m][:, 1 + NS : 1 + NS + 256], in_=ps[:, :256]
                )
                csm = sb2.tile([P, 1 + NS], F32, tag="csm")
                nc.sync.dma_start(out=csm[:], in_=t_cs[:, m, :])
                nc.vector.tensor_copy(out=xT[m][:, : 1 + NS], in_=csm[:])
                nc.vector.memset(xT[m][:, S:SW], 0.0)

            def rmsnorm(dst):
                ssq = pssm.tile([P, SW], F32, tag="pssm")
                for m in range(KT):
                    sqm = sb2.tile([P, SW], F32R, tag="sq")
                    nc.vector.tensor_tensor(
                        out=sqm[:], in0=xT[m][:], in1=xT[m][:], op=AluOpType.mult
                    )
                    nc.tensor.matmul(
                        out=ssq[0:1, :],
                        lhsT=ones_sb[:],
                        rhs=sqm[:],
                        start=(m == 0),
                        stop=(m == KT - 1),
                    )
                lnv = sb2.tile([1, SW], F32, tag="lnv")
                nc.scalar.activation(
                    out=lnv[:], in_=ssq[0:1, :], func=AF.Ln,
                    bias=eps128[0:1, :], scale=1.0 / D,
                )
                rstd = sb2.tile([1, SW], F32, tag="rstd")
                nc.scalar.activation(out=rstd[:], in_=lnv[:], func=AF.Exp, scale=-0.5)
                zps = pssm.tile([P, SW], F32, tag="pssm")
                nc.tensor.matmul(
                    out=zps[:], lhsT=ones_row[:], rhs=rstd[:], start=True, stop=True
                )
                nc.scalar.activation(out=zbuf[:], in_=zps[:], func=AF.Copy)
                for m in range(KT):
                    nc.vector.tensor_tensor(
                        out=dst[m][:], in0=xT[m][:], in1=zbuf[:], op=AluOpType.mult
                    )

            def load_half(tdram, l, mh, pool):
                w = pool.tile([P, KT * 512], F32R, tag="wmat")
                nc.sync.dma_start(
                    out=w[:].rearrange("p (k c) -> p k c", k=KT),
                    in_=tdram[l][:, :, mh * 512 : (mh + 1) * 512],
                )
                return w

            def load_quarter(tdram, l, mq, pool):
                w = pool.tile([P, KT * 256], F32R, tag="wmat")
                nc.sync.dma_start(
                    out=w[:].rearrange("p (k c) -> p k c", k=KT),
                    in_=tdram[l, mq],
                )
                return w

            for l in range(L):
                # ======== attention ========
                rmsnorm(normT)

                def proj_rope(tdram, dstT):
                    for mh in range(4):
                        w_sb = load_quarter(tdram, l, mh, wpool)
                        for mi in range(2):
                            m = mh * 2 + mi
                            ps = ps261.tile([P, SW], F32, tag="ps261")
                            for k in range(KT):
                                nc.tensor.matmul(
                                    out=ps[:],
                                    lhsT=w_sb[
                                        :, k * 256 + mi * P : k * 256 + (mi + 1) * P
                                    ],
                                    rhs=normT[k][:],
                                    start=(k == 0),
                                    stop=(k == KT - 1),
                                )
                            raw = sb2.tile([P, SW], F32R, tag="rawq")
                            nc.scalar.activation(out=raw[:], in_=ps[:], func=AF.Copy)
                            sw = ps261.tile([P, SW], F32, tag="ps261")
                            nc.tensor.matmul(
                                out=sw[:], lhsT=perm_sb[:], rhs=raw[:],
                                start=True, stop=True,
                            )
                            t1 = sb2.tile([P, SW], F32, tag="t1")
                            nc.vector.tensor_tensor(
                                out=t1[:], in0=raw[:], in1=CC_sb[:], op=AluOpType.mult
                            )
                            t2 = sb2.tile([P, SW], F32, tag="t2")
                            nc.vector.tensor_tensor(
                                out=t2[:], in0=sw[:], in1=SS_sb[:], op=AluOpType.mult
                            )
                            nc.vector.tensor_tensor(
                                out=dstT[m][:], in0=t1[:], in1=t2[:], op=AluOpType.add
                            )

                proj_rope(t_wq, qTr)
                proj_rope(t_wk, kTr)

                for mh in range(4):
                    wv_sb = load_quarter(t_wv, l, mh, wpool)
                    for si, (s0, sl) in enumerate(ST):
                        vx = vext[si]
                        if mh == 0:
                            nc.vector.memset(
                                vx[:].bitcast(F32).rearrange(
                                    "p (h c) -> p h c", c=HD + 1
                                )[:, :, HD:],
                                1.0,
                            )
                        ps = ps512.tile([P, 512], F32, tag="ps512")
                        for k in range(KT):
                            nc.tensor.matmul(
                                out=ps[:sl, :256],
                                lhsT=normT[k][:, s0 : s0 + sl],
                                rhs=wv_sb[:, k * 256 : (k + 1) * 256],
                                start=(k == 0),
                                stop=(k == KT - 1),
                            )
                        nc.vector.tensor_copy(
                            out=vx[:sl].rearrange("p (h c) -> p h c", c=HD + 1)[
                                :, mh * 4 : (mh + 1) * 4, :HD
                            ],
                            in_=ps[:sl, :256].rearrange("p (h c) -> p h c", c=HD),
                        )

                # per head: scores->exp->attn+denom->normalize
                for h in range(H):
                    j, hh = divmod(h, 2)
                    tq = qTr[j][hh * HD : (hh + 1) * HD, :]
                    tk = kTr[j][hh * HD : (hh + 1) * HD, :]
                    expts = []
                    dps = pssm.tile([P, SW], F32, tag="pssm")
                    aps = ps261.tile([P, SW], F32, tag="ps261")
                    for si, (s0, sl) in enumerate(ST):
                        sps = ps261.tile([P, SW], F32, tag="ps261")
                        nc.tensor.matmul(
                            out=sps[:sl, :],
                            lhsT=tk[:, s0 : s0 + sl],
                            rhs=tq,
                            start=True,
                            stop=True,
                        )
                        et = sbe.tile([P, SW], F32R, tag="expT")
                        nc.scalar.activation(
                            out=et[:sl, :], in_=sps[:sl, :], func=AF.Exp,
                            scale=float(1.0 / np.sqrt(HD)),
                        )
                        expts.append(et)
                    for si, (s0, sl) in enumerate(ST):
                        nc.tensor.matmul(
                            out=dps[0:1, :],
                            lhsT=vext[si][
                                :sl, h * (HD + 1) + HD : (h + 1) * (HD + 1)
                            ],
                            rhs=expts[si][:sl, :],
                            start=(si == 0),
                            stop=(si == 2),
                        )
                        nc.tensor.matmul(
                            out=aps[0:HD, :],
                            lhsT=vext[si][:sl, h * (HD + 1) : h * (HD + 1) + HD],
                            rhs=expts[si][:sl, :],
                            start=(si == 0),
                            stop=(si == 2),
                        )
                    lnd = sb2.tile([1, SW], F32, tag="lnd")
                    nc.scalar.activation(
                        out=lnd[:], in_=dps[0:1, :], func=AF.Ln,
                        bias=sink_sb[0:1, l * H + h : l * H + h + 1],
                    )
                    z = sb2.tile([1, SW], F32, tag="z")
                    nc.scalar.activation(out=z[:], in_=lnd[:], func=AF.Exp, scale=-1.0)
                    zdp = pssm.tile([P, SW], F32, tag="pssm")
                    nc.tensor.matmul(
                        out=zdp[0:HD, :], lhsT=ones_row[0:1, 0:HD], rhs=z[:],
                        start=True, stop=True,
                    )
                    zd = sb2.tile([P, SW], F32, tag="zd")
                    nc.scalar.activation(out=zd[0:HD, :], in_=zdp[0:HD, :], func=AF.Copy)
                    if hh == 0:
                        nc.vector.tensor_tensor(
                            out=attnT[j][0:HD, :], in0=aps[0:HD, :],
                            in1=zd[0:HD, :], op=AluOpType.mult,
                        )
                    else:
                        tmp = sb2.tile([P, SW], F32R, tag="atmp")
                        nc.vector.tensor_tensor(
                            out=tmp[0:HD, :], in0=aps[0:HD, :],
                            in1=zd[0:HD, :], op=AluOpType.mult,
                        )
                        nc.sync.dma_start(out=attnT[j][HD:P, :], in_=tmp[0:HD, :])

                # wo + residual
                for mh in range(4):
                    wo_sb = load_quarter(t_wo, l, mh, wpool)
                    for mi in range(2):
                        m = mh * 2 + mi
                        ps = ps261.tile([P, SW], F32, tag="ps261")
                        for k in range(KT):
                            nc.tensor.matmul(
                                out=ps[:],
                                lhsT=wo_sb[
                                    :, k * 256 + mi * P : k * 256 + (mi + 1) * P
                                ],
                                rhs=attnT[k][:],
                                start=(k == 0),
                                stop=(k == KT - 1),
                            )
                        nc.vector.tensor_tensor(
                            out=xT[m][:], in0=xT[m][:], in1=ps[:], op=AluOpType.add
                        )

                # ======== MoE ========
                rmsnorm(normT)
                rt_sb = sb2.tile([P, KT * E], F32R, tag="rt")
                nc.sync.dma_start(
                    out=rt_sb[:].rearrange("p (k c) -> p k c", k=KT), in_=t_rt[l]
                )
                gt = sb2.tile([E, SW], F32R, tag="gt")
                for si, (s0, sl) in enumerate(ST):
                    lg_ps = pssm.tile([P, SW], F32, tag="pssm")
                    for k in range(KT):
                        nc.tensor.matmul(
                            out=lg_ps[:sl, :E],
                            lhsT=normT[k][:, s0 : s0 + sl],
                            rhs=rt_sb[:, 0 : KT * E].rearrange(
                                "p (k c) -> p k c", k=KT
                            )[:, k, :],
                            start=(k == 0),
                            stop=(k == KT - 1),
                        )
                    lgn = sb2.tile([P, E], F32, tag="lgn")
                    nc.vector.tensor_copy(out=lgn[:sl, :], in_=lg_ps[:sl, :E])
                    m1 = sb2.tile([P, 1], F32, tag="m1")
                    nc.vector.tensor_reduce(
                        out=m1[:sl, :], in_=lgn[:sl, :],
                        axis=mybir.AxisListType.X, op=AluOpType.max,
                    )
                    mask1 = sb2.tile([P, E], F32, tag="mask1")
                    nc.vector.tensor_scalar(
                        out=mask1[:sl, :], in0=lgn[:sl, :], scalar1=m1[:sl, :],
                        scalar2=None, op0=AluOpType.is_equal,
                    )
                    l2t = sb2.tile([P, E], F32, tag="l2t")
                    nc.vector.tensor_scalar(
                        out=l2t[:sl, :], in0=mask1[:sl, :], scalar1=-1e30,
                        scalar2=None, op0=AluOpType.mult,
                    )
                    nc.vector.tensor_tensor(
                        out=l2t[:sl, :], in0=l2t[:sl, :], in1=lgn[:sl, :],
                        op=AluOpType.add,
                    )
                    m2 = sb2.tile([P, 1], F32, tag="m2")
                    nc.vector.tensor_reduce(
                        out=m2[:sl, :], in_=l2t[:sl, :],
                        axis=mybir.AxisListType.X, op=AluOpType.max,
                    )
                    mask2 = sb2.tile([P, E], F32, tag="mask2")
                    nc.vector.tensor_scalar(
                        out=mask2[:sl, :], in0=l2t[:sl, :], scalar1=m2[:sl, :],
                        scalar2=None, op0=AluOpType.is_equal,
                    )
                    diff = sb2.tile([P, 1], F32, tag="diff")
                    nc.vector.tensor_tensor(
                        out=diff[:sl, :], in0=m2[:sl, :], in1=m1[:sl, :],
                        op=AluOpType.subtract,
                    )
                    gsec = sb2.tile([P, 1], F32, tag="gsec")
                    nc.scalar.activation(out=gsec[:sl, :], in_=diff[:sl, :], func=AF.Sigmoid)
                    gpri = sb2.tile([P, 1], F32, tag="gpri")
                    nc.vector.tensor_scalar(
                        out=gpri[:sl, :], in0=gsec[:sl, :], scalar1=-1.0,
                        scalar2=1.0, op0=AluOpType.mult, op1=AluOpType.add,
                    )
                    gtn = sb2.tile([P, E], F32, tag="gtn")
                    nc.vector.tensor_scalar(
                        out=gtn[:sl, :], in0=mask1[:sl, :], scalar1=gpri[:sl, :],
                        scalar2=None, op0=AluOpType.mult,
                    )
                    gtn2 = sb2.tile([P, E], F32, tag="gtn2")
                    nc.vector.tensor_scalar(
                        out=gtn2[:sl, :], in0=mask2[:sl, :], scalar1=gsec[:sl, :],
                        scalar2=None, op0=AluOpType.mult,
                    )
                    nc.vector.tensor_tensor(
                        out=gtn[:sl, :], in0=gtn[:sl, :], in1=gtn2[:sl, :],
                        op=AluOpType.add,
                    )
                    # transpose [sl, 8] -> [8, sl] into gt columns
                    gtp = pssm.tile([P, SW], F32, tag="pssm")
                    nc.tensor.transpose(
                        out=gtp[:E, :sl], in_=gtn[:sl, :E],
                        identity=ident[:sl, :sl],
                    )
                    nc.vector.tensor_copy(
                        out=gt[:, s0 : s0 + sl], in_=gtp[:E, :sl]
                    )

                # AllGather [normT; gateT]
                for k in range(KT):
                    nc.sync.dma_start(
                        out=agin[k * P : (k + 1) * P, :], in_=normT[k][:, 0:S]
                    )
                nc.sync.dma_start(out=agin[D : D + E, :], in_=gt[:, 0:S])
                agout = agouts[l]
                nc.gpsimd.collective_compute(
                    "AllGather",
                    mybir.AluOpType.bypass,
                    ins=[agin[:].opt()],
                    outs=[agout[:].opt()],
                    replica_groups=RG,
                )

                # my expert's gate rows via indirect row gather + flatten
                goffs_sb = sb2.tile([8, 1], mybir.dt.int32, tag="goffs")
                nc.sync.dma_start(out=goffs_sb[:], in_=t_goffs[:])
                gpart = sb2.tile([8, SW], F32R, tag="gpart")
                nc.gpsimd.indirect_dma_start(
                    out=gpart[:, 0:S],
                    out_offset=None,
                    in_=agout[:],
                    in_offset=bass.IndirectOffsetOnAxis(ap=goffs_sb[:, :1], axis=0),
                )
                nc.sync.dma_start(
                    out=gall[0:1, :].rearrange("p (b s) -> p b s", b=B),
                    in_=gpart[:],
                )

                for bp in range(4):  # block pairs
                    hnbt = []
                    for bi in range(2):
                        b = bp * 2 + bi
                        row = []
                        for k in range(KT):
                            hk = hnb.tile([P, SW], F32R, tag="hnb",
                                          name=f"hn{l}_{b}_{k}")
                            nc.sync.dma_start(
                                out=hk[:, 0:S],
                                in_=agout[
                                    b * (D + E) + k * P : b * (D + E) + (k + 1) * P,
                                    :,
                                ],
                            )
                            row.append(hk)
                        hnbt.append(row)
                    gated = [
                        [
                            gpool.tile([P, SW], F32R, tag=f"gated{bi}_{m}",
                                       name=f"gated{l}_{bp}_{bi}_{m}")
                            for m in range(FT)
                        ]
                        for bi in range(2)
                    ]
                    for q in range(4):
                        w1_sb = load_quarter(t_w1, l, q, epool)
                        w3_sb = load_quarter(t_w3, l, q, epool)
                        for bi in range(2):
                            for mi in range(2):
                                m = q * 2 + mi
                                ps1 = ps261.tile([P, SW], F32, tag="ps261")
                                for k in range(KT):
                                    nc.tensor.matmul(
                                        out=ps1[:],
                                        lhsT=w1_sb[
                                            :, k * 256 + mi * P : k * 256 + (mi + 1) * P
                                        ],
                                        rhs=hnbt[bi][k][:],
                                        start=(k == 0),
                                        stop=(k == KT - 1),
                                    )
                                sil = sb2.tile([P, SW], F32, tag="sil")
                                nc.scalar.activation(
                                    out=sil[:], in_=ps1[:], func=AF.Silu
                                )
                                ps3 = ps261.tile([P, SW], F32, tag="ps261")
                                for k in range(KT):
                                    nc.tensor.matmul(
                                        out=ps3[:],
                                        lhsT=w3_sb[
                                            :, k * 256 + mi * P : k * 256 + (mi + 1) * P
                                        ],
                                        rhs=hnbt[bi][k][:],
                                        start=(k == 0),
                                        stop=(k == KT - 1),
                                    )
                                nc.vector.tensor_tensor(
                                    out=gated[bi][m][:], in0=ps3[:], in1=sil[:],
                                    op=AluOpType.mult,
                                )
                    zgs = []
                    for bi in range(2):
                        b = bp * 2 + bi
                        zgp = pssm.tile([P, SW], F32, tag="pssm")
                        nc.tensor.matmul(
                            out=zgp[:],
                            lhsT=ones_rowr[:],
                            rhs=gall[0:1, b * SW : (b + 1) * SW],
                            start=True, stop=True,
                        )
                        zg = sb2.tile([P, SW], F32, tag="zg")
                        nc.scalar.activation(out=zg[:], in_=zgp[:], func=AF.Copy)
                        zgs.append(zg)
                    for grp in range(4):
                        w2_sb = load_quarter(t_w2, l, grp, epool)
                        for bi in range(2):
                            b = bp * 2 + bi
                            pss = [
                                ps261.tile([P, SW], F32, tag="ps261",
                                           name=f"yps{l}_{b}_{grp}_{i}")
                                for i in range(2)
                            ]
                            for mi in range(2):
                                for k in range(FT):
                                    nc.tensor.matmul(
                                        out=pss[mi][:],
                                        lhsT=w2_sb[
                                            :, k * 256 + mi * P : k * 256 + (mi + 1) * P
                                        ],
                                        rhs=gated[bi][k][:],
                                        start=(k == 0),
                                        stop=(k == FT - 1),
                                    )
                            for mi in range(2):
                                m = grp * 2 + mi
                                yb = sb2.tile([P, SW], BF16, tag="yb")
                                nc.vector.tensor_tensor(
                                    out=yb[:], in0=pss[mi][:], in1=zgs[bi][:],
                                    op=AluOpType.mult,
                                )
                                nc.sync.dma_start(
                                    out=rsin[b * D + m * P : b * D + (m + 1) * P, :],
                                    in_=yb[:, 0:S],
                                )

                nc.gpsimd.collective_compute(
                    "ReduceScatter",
                    mybir.AluOpType.add,
                    ins=[rsin[:].opt()],
                    outs=[rsout[:].opt()],
                    replica_groups=RG,
                )
                for m in range(KT):
                    yk = sb2.tile([P, SW], BF16, tag="yk")
                    nc.sync.dma_start(
                        out=yk[:, 0:S], in_=rsout[m * P : (m + 1) * P, :]
                    )
                    nc.vector.tensor_tensor(
                        out=xT[m][:, 0:S], in0=xT[m][:, 0:S], in1=yk[:, 0:S],
                        op=AluOpType.add,
                    )

            # ---- final norm + transpose out ----
            out_sb = [sb.tile([P, D], F32, tag=f"out{i}", name=f"osb{i}") for i in range(3)]
            ssqf = pssm.tile([P, SW], F32, tag="pssm")
            for m in range(KT):
                sqf = sb2.tile([P, SW], F32R, tag="sq")
                nc.vector.tensor_tensor(
                    out=sqf[:], in0=xT[m][:], in1=xT[m][:], op=AluOpType.mult
                )
                nc.tensor.matmul(
                    out=ssqf[0:1, :], lhsT=ones_sb[:], rhs=sqf[:],
                    start=(m == 0), stop=(m == KT - 1),
                )
            lnf = sb2.tile([1, SW], F32, tag="lnv")
            nc.scalar.activation(
                out=lnf[:], in_=ssqf[0:1, :], func=AF.Ln, bias=eps128[0:1, :],
                scale=1.0 / D,
            )
            rstdf = sb2.tile([1, SW], F32, tag="rstdf")
            nc.scalar.activation(out=rstdf[:], in_=lnf[:], func=AF.Exp, scale=-0.5)
            zpsf = pssm.tile([P, SW], F32, tag="pssm")
            nc.tensor.matmul(
                out=zpsf[:], lhsT=ones_row[:], rhs=rstdf[:], start=True, stop=True
            )
            nc.scalar.activation(out=zbuf[:], in_=zpsf[:], func=AF.Copy)
            for m in range(KT):
                xnf = sb2.tile([P, SW], F32, tag="xnf")
                nc.vector.tensor_tensor(
                    out=xnf[:], in0=xT[m][:], in1=zbuf[:], op=AluOpType.mult
                )
                for si, (s0, sl) in enumerate(ST):
                    tp = ps261.tile([P, SW], F32, tag="ps261")
                    nc.tensor.transpose(
                        out=tp[:sl, :P], in_=xnf[:, s0 : s0 + sl], identity=ident[:]
                    )
                    nc.vector.tensor_copy(
                        out=out_sb[si][:sl, m * P : (m + 1) * P], in_=tp[:sl, :P]
                    )
            for si, (s0, sl) in enumerate(ST):
                nc.sync.dma_start(out=t_out[s0 : s0 + sl, :], in_=out_sb[si][:sl, :])

    nc.compile()
    return nc


_NC_CACHE = None


def kernel(**inputs):
    global LAST_RESULT, _NC_CACHE
    prep = _host_prep(inputs)
    in_maps = []
    for b in range(B):
        patches = np.asarray(inputs["patches"][b]).astype(np.float32)
        pT = np.ascontiguousarray(patches.T.reshape(PT, P, 256))
        in_maps.append(
            {
                "patchesT": pT,
                "w_img": prep["w_img"],
                "clsstorT": prep["clsstorT"],
                "CC": prep["CC"],
                "SS": prep["SS"],
                "perm": prep["perm"],
                "expsink": prep["expsink"],
                "wq": prep["wq"],
                "wk": prep["wk"],
                "wv": prep["wv"],
                "wo": prep["wo"],
                "rt": prep["rt"],
                "w1": np.ascontiguousarray(prep["w1"][:, b]),
                "w3": np.ascontiguousarray(prep["w3"][:, b]),
                "w2": np.ascontiguousarray(prep["w2"][:, b]),
                "goffs": np.array(
                    [[bb * (D + E) + D + b] for bb in range(B)], np.int32
                ),
            }
        )
    if _NC_CACHE is None:
        _NC_CACHE = build_nc()
    nc = _NC_CACHE
    trace = bool(int(os.environ.get("KERNEL_TRACE", "0")))
    if trace:
        try:
            import axon_profile_hook

            axon_profile_hook.install()
        except Exception:
            pass
    res = run_bass_kernel_spmd(nc, in_maps, core_ids=list(range(B)), trace=trace)
    LAST_RESULT = res
    out = np.stack([res.results[b]["out"] for b in range(B)], 0)
    final_w = np.asarray(inputs["final_norm_w"]).astype(np.float32)
    return out * final_w[None, None, :]


# revision 21
# speedup vs baseline: 1.1433x; 1.0051x over previous
"""Trainium2 Bass kernel for nn_AMOE_79843442033161 (ViT-MoE, 4 layers).

Sharding: data-parallel attention (1 image per core, 8 cores) + expert-parallel
MoE (1 expert per core, dense tokens): per layer, AllGather transposed
activations+gates, each core runs its expert over all 2088 tokens,
ReduceScatter (bf16) combines expert outputs back to token owners.

All matmuls fp32r (f32 bytes, full PE rate): requires even moving-N (pad
261->262) and dst start_partition 0. Residual transposed xT [1024, 262] f32
(col 261 junk). Norm weights folded into following matmuls host-side; final
norm weight applied on host.
"""
import os
import sys
import numpy as np

sys.path.insert(0, "/opt/trn_rl_repo")

import concourse.bass as bass
import concourse.mybir as mybir
import concourse.tile as tile
from concourse import bacc, bass_isa
from concourse.bass_utils import run_bass_kernel_spmd
from concourse.alu_op_type import AluOpType
from concourse.masks import make_identity

B, S, D, H, HD, L, E, F = 8, 261, 1024, 16, 64, 4, 8, 1024
SW = 262  # padded token width (fp32r needs even moving-N)
NS = 4
PDIM = 768
EPS = 1e-6
THETA = 10000.0
P = 128
KT = D // P
FT = F // P
PT = PDIM // P
ST = [(0, 128), (128, 128), (256, 5)]
F32 = mybir.dt.float32
F32R = mybir.dt.float32r
BF16 = mybir.dt.bfloat16
AF = mybir.ActivationFunctionType

LAST_RESULT = None


def _host_prep(inputs):
    lp = {k: np.asarray(v) for k, v in inputs["layer_params"].items()}
    out = {}

    def tile_w(w):  # [Din, Dout] f32 -> [128(p), kt, Dout]
        din, dout = w.shape
        kt = din // P
        return np.ascontiguousarray(
            w.astype(np.float32).reshape(kt, P, dout).transpose(1, 0, 2)
        )

    def tile_w4(w):  # [Din, Dout] -> [4(q), 128(p), kt, Dout//4] contiguous
        t = tile_w(w)  # [P, kt, Dout]
        p_, kt_, dout = t.shape
        return np.ascontiguousarray(
            t.reshape(p_, kt_, 4, dout // 4).transpose(2, 0, 1, 3)
        )

    wq = np.empty((L, 4, P, KT, D // 4), np.float32)
    wk = np.empty((L, 4, P, KT, D // 4), np.float32)
    wv = np.empty((L, 4, P, KT, D // 4), np.float32)
    wo = np.empty((L, 4, P, KT, D // 4), np.float32)
    rt = np.empty((L, P, KT, E), np.float32)
    w1 = np.empty((L, E, 4, P, KT, F // 4), np.float32)
    w3 = np.empty((L, E, 4, P, KT, F // 4), np.float32)
    w2 = np.empty((L, E, 4, P, FT, D // 4), np.float32)
    for l in range(L):
        an = lp["attn_norm"][l][:, None].astype(np.float32)
        fn = lp["ffn_norm"][l][:, None].astype(np.float32)
        wq[l] = tile_w4(an * lp["wq"][l])
        wk[l] = tile_w4(an * lp["wk"][l])
        wv[l] = tile_w4(an * lp["wv"][l])
        wo[l] = tile_w4(lp["wo"][l])
        rt[l] = tile_w(fn * lp["router"][l])
        for e in range(E):
            w1[l, e] = tile_w4(fn * lp["w1"][l, e])
            w3[l, e] = tile_w4(fn * lp["w3"][l, e])
            w2[l, e] = tile_w4(lp["w2"][l, e])
    out.update(wq=wq, wk=wk, wv=wv, wo=wo, rt=rt, w1=w1, w3=w3, w2=w2)
    out["expsink"] = np.ascontiguousarray(
        np.exp(lp["sink"]).astype(np.float32).reshape(1, L * H)
    )

    w_img = np.asarray(inputs["w_img"]).astype(np.float32)
    out["w_img"] = np.ascontiguousarray(w_img.reshape(PT, P, D).transpose(1, 0, 2))
    cs = np.concatenate(
        [
            np.asarray(inputs["cls_token"]).reshape(1, D),
            np.asarray(inputs["storage"]).reshape(NS, D),
        ],
        0,
    ).astype(np.float32)
    out["clsstorT"] = np.ascontiguousarray(
        cs.T.reshape(KT, P, 1 + NS).transpose(1, 0, 2)
    )

    inv = 1.0 / (THETA ** (np.arange(0, HD, 2, dtype=np.float32) / HD))
    fr = np.outer(np.arange(S, dtype=np.float32), inv)
    cos, sin = np.cos(fr), np.sin(fr)
    CC = np.zeros((P, SW), np.float32)
    SS = np.zeros((P, SW), np.float32)
    for p in range(P):
        d = p % HD
        j = d // 2
        CC[p, :S] = cos[:, j]
        SS[p, :S] = (1.0 if d % 2 == 1 else -1.0) * sin[:, j]
    out["CC"] = CC
    out["SS"] = SS
    perm = np.zeros((P, P), np.float32)
    for p in range(P):
        perm[p ^ 1, p] = 1.0
    out["perm"] = perm
    return out


def build_nc():
    nc = bacc.Bacc("TRN2", target_bir_lowering=False, debug=False, num_devices=B)
    dt = nc.dram_tensor
    t_patches = dt("patchesT", [PT, P, 256], F32R, kind="ExternalInput")
    t_wimg = dt("w_img", [P, PT, D], F32R, kind="ExternalInput")
    t_cs = dt("clsstorT", [P, KT, 1 + NS], F32, kind="ExternalInput")
    t_CC = dt("CC", [P, SW], F32, kind="ExternalInput")
    t_SS = dt("SS", [P, SW], F32, kind="ExternalInput")
    t_perm = dt("perm", [P, P], F32R, kind="ExternalInput")
    t_sink = dt("expsink", [1, L * H], F32, kind="ExternalInput")
    t_wq = dt("wq", [L, 4, P, KT, D // 4], F32R, kind="ExternalInput")
    t_wk = dt("wk", [L, 4, P, KT, D // 4], F32R, kind="ExternalInput")
    t_wv = dt("wv", [L, 4, P, KT, D // 4], F32R, kind="ExternalInput")
    t_wo = dt("wo", [L, 4, P, KT, D // 4], F32R, kind="ExternalInput")
    t_rt = dt("rt", [L, P, KT, E], F32R, kind="ExternalInput")
    t_w1 = dt("w1", [L, 4, P, KT, F // 4], F32R, kind="ExternalInput")
    t_w3 = dt("w3", [L, 4, P, KT, F // 4], F32R, kind="ExternalInput")
    t_w2 = dt("w2", [L, 4, P, FT, D // 4], F32R, kind="ExternalInput")
    t_out = dt("out", [S, D], F32, kind="ExternalOutput")
    t_goffs = dt("goffs", [8, 1], mybir.dt.int32, kind="ExternalInput")

    RG = [list(range(B))]

    with tile.TileContext(nc) as tc:
        import contextlib

        with contextlib.ExitStack() as ctx:
            sb = ctx.enter_context(tc.tile_pool(name="sb", bufs=1))
            sb2 = ctx.enter_context(tc.tile_pool(name="sb2", bufs=2))
            sbe = ctx.enter_context(tc.tile_pool(name="sbe", bufs=6))
            gpool = ctx.enter_context(tc.tile_pool(name="gpool", bufs=2))
            hnb = ctx.enter_context(tc.tile_pool(name="hnb", bufs=18))
            wpool = ctx.enter_context(tc.tile_pool(name="wpool", bufs=2))
            epool = ctx.enter_context(tc.tile_pool(name="epool", bufs=2))
            ps261 = ctx.enter_context(tc.tile_pool(name="ps261", bufs=4, space="PSUM"))
            ps512 = ctx.enter_context(tc.tile_pool(name="ps512", bufs=2, space="PSUM"))
            pssm = ctx.enter_context(tc.tile_pool(name="pssm", bufs=2, space="PSUM"))
            dram = ctx.enter_context(tc.tile_pool(name="dram", bufs=1, space="DRAM"))

            CC_sb = sb.tile([P, SW], F32, tag="CC")
            SS_sb = sb.tile([P, SW], F32, tag="SS")
            perm_sb = sb.tile([P, P], F32R, tag="perm")
            sink_sb = sb.tile([1, L * H], F32, tag="sink")
            ones_sb = sb.tile([P, 1], F32R, tag="ones")
            ones_row = sb.tile([1, P], F32, tag="onesrow")
            ones_rowr = sb.tile([1, P], F32R, tag="onesrowr")
            ident = sb.tile([P, P], F32, tag="ident")
            nc.sync.dma_start(out=CC_sb[:], in_=t_CC[:])
            nc.sync.dma_start(out=SS_sb[:], in_=t_SS[:])
            nc.sync.dma_start(out=perm_sb[:], in_=t_perm[:])
            nc.sync.dma_start(out=sink_sb[:], in_=t_sink[:])
            nc.vector.memset(ones_sb[:].bitcast(F32), 1.0)
            nc.vector.memset(ones_row[:], 1.0)
            nc.vector.memset(ones_rowr[:].bitcast(F32), 1.0)
            eps128 = sb.tile([P, 1], F32, tag="eps")
            nc.vector.memset(eps128[:], EPS)
            make_identity(nc, ident[:])

            xT = [sb.tile([P, SW], F32, tag=f"xT{m}", name=f"xT{m}") for m in range(KT)]
            normT = [sb.tile([P, SW], F32R, tag=f"nT{m}", name=f"nT{m}") for m in range(KT)]
            qTr = [sb.tile([P, SW], F32R, tag=f"qTr{m}", name=f"qTr{m}") for m in range(KT)]
            kTr = [sb.tile([P, SW], F32R, tag=f"kTr{m}", name=f"kTr{m}") for m in range(KT)]
            attnT = [sb.tile([P, SW], F32R, tag=f"aT{m}", name=f"aT{m}") for m in range(KT)]
            vext = [sb.tile([P, H * (HD + 1)], F32R, tag=f"vx{i}", name=f"vx{i}") for i in range(3)]
            zbuf = sb.tile([P, SW], F32, tag="zbuf")
            gall = sb.tile([1, B * SW], F32R, tag="gall")

            agin = dram.tile([D + E, S], F32R)
            agouts = [
                dram.tile([B * (D + E), S], F32R, addr_space="Shared",
                          tag=f"agout{i}", name=f"agout{i}")
                for i in range(L)
            ]
            rsin_a = dram.tile([B * (D // 2), S], BF16)
            rsin_b = dram.tile([B * (D // 2), S], BF16)
            rsout_a = dram.tile([D // 2, S], BF16)
            rsout_b = dram.tile([D // 2, S], BF16)

            # ---- embed ----
            for m in range(KT):
                ps = ps512.tile([P, 512], F32, tag="ps512")
                for k in range(PT):
                    wi = wpool.tile([P, P], F32R, tag="wmat")
                    nc.sync.dma_start(out=wi[:], in_=t_wimg[:, k, m * P : (m + 1) * P])
                    pk = hnb.tile([P, 256], F32R, tag="hnb")
                    nc.sync.dma_start(out=pk[:], in_=t_patches[k])
                    nc.tensor.matmul(
                        out=ps[:, :256],
                        lhsT=wi[:],
                        rhs=pk[:],
                        start=(k == 0),
                        stop=(k == PT - 1),
                    )
                nc.vector.tensor_copy(
                    out=xT[m][:, 1 + NS : 1 + NS + 256], in_=ps[:, :256]
                )
                csm = sb2.tile([P, 1 + NS], F32, tag="csm")
                nc.sync.dma_start(out=csm[:], in_=t_cs[:, m, :])
                nc.vector.tensor_copy(out=xT[m][:, : 1 + NS], in_=csm[:])
                nc.vector.memset(xT[m][:, S:SW], 0.0)

            def rmsnorm(dst):
                ssq = pssm.tile([P, SW], F32, tag="pssm")
                for m in range(KT):
                    sqm = sb2.tile([P, SW], F32R, tag="sq")
                    nc.vector.tensor_tensor(
                        out=sqm[:], in0=xT[m][:], in1=xT[m][:], op=AluOpType.mult
                    )
                    nc.tensor.matmul(
                        out=ssq[0:1, :],
                        lhsT=ones_sb[:],
                        rhs=sqm[:],
                        start=(m == 0),
                        stop=(m == KT - 1),
                    )
                lnv = sb2.tile([1, SW], F32, tag="lnv")
                nc.scalar.activation(
                    out=lnv[:], in_=ssq[0:1, :], func=AF.Ln,
                    bias=eps128[0:1, :], scale=1.0 / D,
                )
                rstd = sb2.tile([1, SW], F32, tag="rstd")
                nc.scalar.activation(out=rstd[:], in_=lnv[:], func=AF.Exp, scale=-0.5)
                zps = pssm.tile([P, SW], F32, tag="pssm")
                nc.tensor.matmul(
                    out=zps[:], lhsT=ones_row[:], rhs=rstd[:], start=True, stop=True
                )
                nc.scalar.activation(out=zbuf[:], in_=zps[:], func=AF.Copy)
                for m in range(KT):
                    nc.vector.tensor_tensor(
                        out=dst[m][:], in0=xT[m][:], in1=zbuf[:], op=AluOpType.mult
                    )

            def load_half(tdram, l, mh, pool):
                w = pool.tile([P, KT * 512], F32R, tag="wmat")
                nc.sync.dma_start(
                    out=w[:].rearrange("p (k c) -> p k c", k=KT),
                    in_=tdram[l][:, :, mh * 512 : (mh + 1) * 512],
                )
                return w

            def load_quarter(tdram, l, mq, pool):
                w = pool.tile([P, KT * 256], F32R, tag="wmat")
                nc.sync.dma_start(
                    out=w[:].rearrange("p (k c) -> p k c", k=KT),
                    in_=tdram[l, mq],
                )
                return w

            for l in range(L):
                # ======== attention ========
                rmsnorm(normT)

                def proj_rope(tdram, dstT):
                    for mh in range(4):
                        w_sb = load_quarter(tdram, l, mh, wpool)
                        for mi in range(2):
                            m = mh * 2 + mi
                            ps = ps261.tile([P, SW], F32, tag="ps261")
                            for k in range(KT):
                                nc.tensor.matmul(
                                    out=ps[:],
                                    lhsT=w_sb[
                                        :, k * 256 + mi * P : k * 256 + (mi + 1) * P
                                    ],
                                    rhs=normT[k][:],
                                    start=(k == 0),
                                    stop=(k == KT - 1),
                                )
                            raw = sb2.tile([P, SW], F32R, tag="rawq")
                            nc.scalar.activation(out=raw[:], in_=ps[:], func=AF.Copy)
                            sw = ps261.tile([P, SW], F32, tag="ps261")
                            nc.tensor.matmul(
                                out=sw[:], lhsT=perm_sb[:], rhs=raw[:],
                                start=True, stop=True,
                            )
                            t1 = sb2.tile([P, SW], F32, tag="t1")
                            nc.vector.tensor_tensor(
                                out=t1[:], in0=raw[:], in1=CC_sb[:], op=AluOpType.mult
                            )
                            t2 = sb2.tile([P, SW], F32, tag="t2")
                            nc.vector.tensor_tensor(
                                out=t2[:], in0=sw[:], in1=SS_sb[:], op=AluOpType.mult
                            )
                            nc.vector.tensor_tensor(
                                out=dstT[m][:], in0=t1[:], in1=t2[:], op=AluOpType.add
                            )

                proj_rope(t_wq, qTr)
                proj_rope(t_wk, kTr)

                for mh in range(4):
                    wv_sb = load_quarter(t_wv, l, mh, wpool)
                    for si, (s0, sl) in enumerate(ST):
                        vx = vext[si]
                        if mh == 0:
                            nc.vector.memset(
                                vx[:].bitcast(F32).rearrange(
                                    "p (h c) -> p h c", c=HD + 1
                                )[:, :, HD:],
                                1.0,
                            )
                        ps = ps512.tile([P, 512], F32, tag="ps512")
                        for k in range(KT):
                            nc.tensor.matmul(
                                out=ps[:sl, :256],
                                lhsT=normT[k][:, s0 : s0 + sl],
                                rhs=wv_sb[:, k * 256 : (k + 1) * 256],
                                start=(k == 0),
                                stop=(k == KT - 1),
                            )
                        nc.vector.tensor_copy(
                            out=vx[:sl].rearrange("p (h c) -> p h c", c=HD + 1)[
                                :, mh * 4 : (mh + 1) * 4, :HD
                            ],
                            in_=ps[:sl, :256].rearrange("p (h c) -> p h c", c=HD),
                        )

                # per head: scores->exp->attn+denom->normalize
                for h in range(H):
                    j, hh = divmod(h, 2)
                    tq = qTr[j][hh * HD : (hh + 1) * HD, :]
                    tk = kTr[j][hh * HD : (hh + 1) * HD, :]
                    expts = []
                    dps = pssm.tile([P, SW], F32, tag="pssm")
                    aps = ps261.tile([P, SW], F32, tag="ps261")
                    for si, (s0, sl) in enumerate(ST):
                        sps = ps261.tile([P, SW], F32, tag="ps261")
                        nc.tensor.matmul(
                            out=sps[:sl, :],
                            lhsT=tk[:, s0 : s0 + sl],
                            rhs=tq,
                            start=True,
                            stop=True,
                        )
                        et = sbe.tile([P, SW], F32R, tag="expT")
                        nc.scalar.activation(
                            out=et[:sl, :], in_=sps[:sl, :], func=AF.Exp,
                            scale=float(1.0 / np.sqrt(HD)),
                        )
                        expts.append(et)
                    for si, (s0, sl) in enumerate(ST):
                        nc.tensor.matmul(
                            out=dps[0:1, :],
                            lhsT=vext[si][
                                :sl, h * (HD + 1) + HD : (h + 1) * (HD + 1)
                            ],
                            rhs=expts[si][:sl, :],
                            start=(si == 0),
                            stop=(si == 2),
                        )
                        nc.tensor.matmul(
                            out=aps[0:HD, :],
                            lhsT=vext[si][:sl, h * (HD + 1) : h * (HD + 1) + HD],
                            rhs=expts[si][:sl, :],
                            start=(si == 0),
                            stop=(si == 2),
                        )
                    lnd = sb2.tile([1, SW], F32, tag="lnd")
                    nc.scalar.activation(
                        out=lnd[:], in_=dps[0:1, :], func=AF.Ln,
                        bias=sink_sb[0:1, l * H + h : l * H + h + 1],
                    )
                    z = sb2.tile([1, SW], F32, tag="z")
                    nc.scalar.activation(out=z[:], in_=lnd[:], func=AF.Exp, scale=-1.0)
                    zdp = pssm.tile([P, SW], F32, tag="pssm")
                    nc.tensor.matmul(
                        out=zdp[0:HD, :], lhsT=ones_row[0:1, 0:HD], rhs=z[:],
                        start=True, stop=True,
                    )
                    zd = sb2.tile([P, SW], F32, tag="zd")
                    nc.scalar.activation(out=zd[0:HD, :], in_=zdp[0:HD, :], func=AF.Copy)
                    if hh == 0:
                        nc.vector.tensor_tensor(
                            out=attnT[j][0:HD, :], in0=aps[0:HD, :],
                            in1=zd[0:HD, :], op=AluOpType.mult,
                        )
                    else:
                        tmp = sb2.tile([P, SW], F32R, tag="atmp")
                        nc.vector.tensor_tensor(
                            out=tmp[0:HD, :], in0=aps[0:HD, :],
                            in1=zd[0:HD, :], op=AluOpType.mult,
                        )
                        nc.sync.dma_start(out=attnT[j][HD:P, :], in_=tmp[0:HD, :])

                # wo + residual
                for mh in range(4):
                    wo_sb = load_quarter(t_wo, l, mh, wpool)
                    for mi in range(2):
                        m = mh * 2 + mi
                        ps = ps261.tile([P, SW], F32, tag="ps261")
                        for k in range(KT):
                            nc.tensor.matmul(
                                out=ps[:],
                                lhsT=wo_sb[
                                    :, k * 256 + mi * P : k * 256 + (mi + 1) * P
                                ],
                                rhs=attnT[k][:],
                                start=(k == 0),
                                stop=(k == KT - 1),
                            )
                        nc.vector.tensor_tensor(
                            out=xT[m][:], in0=xT[m][:], in1=ps[:], op=AluOpType.add
                        )

                # ======== MoE ========
                rmsnorm(normT)
                rt_sb = sb2.tile([P, KT * E], F32R, tag="rt")
                nc.sync.dma_start(
                    out=rt_sb[:].rearrange("p (k c) -> p k c", k=KT), in_=t_rt[l]
                )
                gt = sb2.tile([E, SW], F32R, tag="gt")
                for si, (s0, sl) in enumerate(ST):
                    lg_ps = pssm.tile([P, SW], F32, tag="pssm")
                    for k in range(KT):
                        nc.tensor.matmul(
                            out=lg_ps[:sl, :E],
                            lhsT=normT[k][:, s0 : s0 + sl],
                            rhs=rt_sb[:, 0 : KT * E].rearrange(
                                "p (k c) -> p k c", k=KT
                            )[:, k, :],
                            start=(k == 0),
                            stop=(k == KT - 1),
                        )
                    lgn = sb2.tile([P, E], F32, tag="lgn")
                    nc.vector.tensor_copy(out=lgn[:sl, :], in_=lg_ps[:sl, :E])
                    m1 = sb2.tile([P, 1], F32, tag="m1")
                    nc.vector.tensor_reduce(
                        out=m1[:sl, :], in_=lgn[:sl, :],
                        axis=mybir.AxisListType.X, op=AluOpType.max,
                    )
                    mask1 = sb2.tile([P, E], F32, tag="mask1")
                    nc.vector.tensor_scalar(
                        out=mask1[:sl, :], in0=lgn[:sl, :], scalar1=m1[:sl, :],
                        scalar2=None, op0=AluOpType.is_equal,
                    )
                    l2t = sb2.tile([P, E], F32, tag="l2t")
                    nc.vector.tensor_scalar(
                        out=l2t[:sl, :], in0=mask1[:sl, :], scalar1=-1e30,
                        scalar2=None, op0=AluOpType.mult,
                    )
                    nc.vector.tensor_tensor(
                        out=l2t[:sl, :], in0=l2t[:sl, :], in1=lgn[:sl, :],
                        op=AluOpType.add,
                    )
                    m2 = sb2.tile([P, 1], F32, tag="m2")
                    nc.vector.tensor_reduce(
                        out=m2[:sl, :], in_=l2t[:sl, :],
                        axis=mybir.AxisListType.X, op=AluOpType.max,
                    )
                    mask2 = sb2.tile([P, E], F32, tag="mask2")
                    nc.vector.tensor_scalar(
                        out=mask2[:sl, :], in0=l2t[:sl, :], scalar1=m2[:sl, :],
                        scalar2=None, op0=AluOpType.is_equal,
                    )
                    diff = sb2.tile([P, 1], F32, tag="diff")
                    nc.vector.tensor_tensor(
                        out=diff[:sl, :], in0=m2[:sl, :], in1=m1[:sl, :],
                        op=AluOpType.subtract,
                    )
                    gsec = sb2.tile([P, 1], F32, tag="gsec")
                    nc.scalar.activation(out=gsec[:sl, :], in_=diff[:sl, :], func=AF.Sigmoid)
                    gpri = sb2.tile([P, 1], F32, tag="gpri")
                    nc.vector.tensor_scalar(
                        out=gpri[:sl, :], in0=gsec[:sl, :], scalar1=-1.0,
                        scalar2=1.0, op0=AluOpType.mult, op1=AluOpType.add,
                    )
                    gtn = sb2.tile([P, E], F32, tag="gtn")
                    nc.vector.tensor_scalar(
                        out=gtn[:sl, :], in0=mask1[:sl, :], scalar1=gpri[:sl, :],
                        scalar2=None, op0=AluOpType.mult,
                    )
                    gtn2 = sb2.tile([P, E], F32, tag="gtn2")
                    nc.vector.tensor_scalar(
                        out=gtn2[:sl, :], in0=mask2[:sl, :], scalar1=gsec[:sl, :],
                        scalar2=None, op0=AluOpType.mult,
                    )
                    nc.vector.tensor_tensor(
                        out=gtn[:sl, :], in0=gtn[:sl, :], in1=gtn2[:sl, :],
                        op=AluOpType.add,
                    )
                    # transpose [sl, 8] -> [8, sl] into gt columns
                    gtp = pssm.tile([P, SW], F32, tag="pssm")
                    nc.tensor.transpose(
                        out=gtp[:E, :sl], in_=gtn[:sl, :E],
                        identity=ident[:sl, :sl],
                    )
                    nc.vector.tensor_copy(
                        out=gt[:, s0 : s0 + sl], in_=gtp[:E, :sl]
                    )

                # AllGather [normT; gateT]
                for k in range(KT):
                    nc.sync.dma_start(
                        out=agin[k * P : (k + 1) * P, :], in_=normT[k][:, 0:S]
                    )
                nc.sync.dma_start(out=agin[D : D + E, :], in_=gt[:, 0:S])
                agout = agouts[l]
                nc.gpsimd.collective_compute(
                    "AllGather",
                    mybir.AluOpType.bypass,
                    ins=[agin[:].opt()],
                    outs=[agout[:].opt()],
                    replica_groups=RG,
                )

                # my expert's gate rows via indirect row gather + flatten
                goffs_sb = sb2.tile([8, 1], mybir.dt.int32, tag="goffs")
                nc.sync.dma_start(out=goffs_sb[:], in_=t_goffs[:])
                gpart = sb2.tile([8, SW], F32R, tag="gpart")
                nc.gpsimd.indirect_dma_start(
                    out=gpart[:, 0:S],
                    out_offset=None,
                    in_=agout[:],
                    in_offset=bass.IndirectOffsetOnAxis(ap=goffs_sb[:, :1], axis=0),
                )
                nc.sync.dma_start(
                    out=gall[0:1, :].rearrange("p (b s) -> p b s", b=B),
                    in_=gpart[:],
                )

                for bp in range(4):  # block pairs
                    hnbt = []
                    for bi in range(2):
                        b = bp * 2 + bi
                        row = []
                        for k in range(KT):
                            hk = hnb.tile([P, SW], F32R, tag="hnb",
                                          name=f"hn{l}_{b}_{k}")
                            nc.sync.dma_start(
                                out=hk[:, 0:S],
                                in_=agout[
                                    b * (D + E) + k * P : b * (D + E) + (k + 1) * P,
                                    :,
                                ],
                            )
                            row.append(hk)
                        hnbt.append(row)
                    gated = [
                        [
                            gpool.tile([P, SW], F32R, tag=f"gated{bi}_{m}",
                                       name=f"gated{l}_{bp}_{bi}_{m}")
                            for m in range(FT)
                        ]
                        for bi in range(2)
                    ]
                    for q in range(4):
                        w1_sb = load_quarter(t_w1, l, q, epool)
                        w3_sb = load_quarter(t_w3, l, q, epool)
                        for bi in range(2):
                            for mi in range(2):
                                m = q * 2 + mi
                                ps1 = ps261.tile([P, SW], F32, tag="ps261")
                                for k in range(KT):
                                    nc.tensor.matmul(
                                        out=ps1[:],
                                        lhsT=w1_sb[
                                            :, k * 256 + mi * P : k * 256 + (mi + 1) * P
                                        ],
                                        rhs=hnbt[bi][k][:],
                                        start=(k == 0),
                                        stop=(k == KT - 1),
                                    )
                                sil = sb2.tile([P, SW], F32, tag="sil")
                                nc.scalar.activation(
                                    out=sil[:], in_=ps1[:], func=AF.Silu
                                )
                                ps3 = ps261.tile([P, SW], F32, tag="ps261")
                                for k in range(KT):
                                    nc.tensor.matmul(
                                        out=ps3[:],
                                        lhsT=w3_sb[
                                            :, k * 256 + mi * P : k * 256 + (mi + 1) * P
                                        ],
                                        rhs=hnbt[bi][k][:],
                                        start=(k == 0),
                                        stop=(k == KT - 1),
                                    )
                                nc.vector.tensor_tensor(
                                    out=gated[bi][m][:], in0=ps3[:], in1=sil[:],
                                    op=AluOpType.mult,
                                )
                    zgs = []
                    for bi in range(2):
                        b = bp * 2 + bi
                        zgp = pssm.tile([P, SW], F32, tag="pssm")
                        nc.tensor.matmul(
                            out=zgp[:],
                            lhsT=ones_rowr[:],
                            rhs=gall[0:1, b * SW : (b + 1) * SW],
                            start=True, stop=True,
                        )
                        zg = sb2.tile([P, SW], F32, tag="zg")
                        nc.scalar.activation(out=zg[:], in_=zgp[:], func=AF.Copy)
                        zgs.append(zg)
                    for grp in range(4):
                        w2_sb = load_quarter(t_w2, l, grp, epool)
                        for bi in range(2):
                            b = bp * 2 + bi
                            pss = [
                                ps261.tile([P, SW], F32, tag="ps261",
                                           name=f"yps{l}_{b}_{grp}_{i}")
                                for i in range(2)
                            ]
                            for mi in range(2):
                                for k in range(FT):
                                    nc.tensor.matmul(
                                        out=pss[mi][:],
                                        lhsT=w2_sb[
                                            :, k * 256 + mi * P : k * 256 + (mi + 1) * P
                                        ],
                                        rhs=gated[bi][k][:],
                                        start=(k == 0),
                                        stop=(k == FT - 1),
                                    )
                            for mi in range(2):
                                m = grp * 2 + mi
                                yb = sb2.tile([P, SW], BF16, tag="yb")
                                nc.vector.tensor_tensor(
                                    out=yb[:], in0=pss[mi][:], in1=zgs[bi][:],
                                    op=AluOpType.mult,
                                )
                                rs_buf = rsin_a if m < 4 else rsin_b
                                mm_ = m % 4
                                nc.sync.dma_start(
                                    out=rs_buf[
                                        b * (D // 2) + mm_ * P : b * (D // 2)
                                        + (mm_ + 1) * P,
                                        :,
                                    ],
                                    in_=yb[:, 0:S],
                                )

                nc.gpsimd.collective_compute(
                    "ReduceScatter",
                    mybir.AluOpType.add,
                    ins=[rsin_a[:].opt()],
                    outs=[rsout_a[:].opt()],
                    replica_groups=RG,
                )
                nc.gpsimd.collective_compute(
                    "ReduceScatter",
                    mybir.AluOpType.add,
                    ins=[rsin_b[:].opt()],
                    outs=[rsout_b[:].opt()],
                    replica_groups=RG,
                )
                for m in range(KT):
                    rs_buf = rsout_a if m < 4 else rsout_b
                    mm_ = m % 4
                    yk = sb2.tile([P, SW], BF16, tag="yk")
                    nc.sync.dma_start(
                        out=yk[:, 0:S], in_=rs_buf[mm_ * P : (mm_ + 1) * P, :]
                    )
                    nc.vector.tensor_tensor(
                        out=xT[m][:, 0:S], in0=xT[m][:, 0:S], in1=yk[:, 0:S],
                        op=AluOpType.add,
                    )

            # ---- final norm + transpose out ----
            out_sb = [sb.tile([P, D], F32, tag=f"out{i}", name=f"osb{i}") for i in range(3)]
            ssqf = pssm.tile([P, SW], F32, tag="pssm")
            for m in range(KT):
                sqf = sb2.tile([P, SW], F32R, tag="sq")
                nc.vector.tensor_tensor(
                    out=sqf[:], in0=xT[m][:], in1=xT[m][:], op=AluOpType.mult
                )
                nc.tensor.matmul(
                    out=ssqf[0:1, :], lhsT=ones_sb[:], rhs=sqf[:],
                    start=(m == 0), stop=(m == KT - 1),
                )
            lnf = sb2.tile([1, SW], F32, tag="lnv")
            nc.scalar.activation(
                out=lnf[:], in_=ssqf[0:1, :], func=AF.Ln, bias=eps128[0:1, :],
                scale=1.0 / D,
            )
            rstdf = sb2.tile([1, SW], F32, tag="rstdf")
            nc.scalar.activation(out=rstdf[:], in_=lnf[:], func=AF.Exp, scale=-0.5)
            zpsf = pssm.tile([P, SW], F32, tag="pssm")
            nc.tensor.matmul(
                out=zpsf[:], lhsT=ones_row[:], rhs=rstdf[:], start=True, stop=True
            )
            nc.scalar.activation(out=zbuf[:], in_=zpsf[:], func=AF.Copy)
            for m in range(KT):
                xnf = sb2.tile([P, SW], F32, tag="xnf")
                nc.vector.tensor_tensor(
                    out=xnf[:], in0=xT[m][:], in1=zbuf[:], op=AluOpType.mult
                )
                for si, (s0, sl) in enumerate(ST):
                    tp = ps261.tile([P, SW], F32, tag="ps261")
                    nc.tensor.transpose(
                        out=tp[:sl, :P], in_=xnf[:, s0 : s0 + sl], identity=ident[:]
                    )
                    nc.vector.tensor_copy(
                        out=out_sb[si][:sl, m * P : (m + 1) * P], in_=tp[:sl, :P]
                    )
            for si, (s0, sl) in enumerate(ST):
                nc.sync.dma_start(out=t_out[s0 : s0 + sl, :], in_=out_sb[si][:sl, :])

    nc.compile()
    return nc


_NC_CACHE = None


def kernel(**inputs):
    global LAST_RESULT, _NC_CACHE
    prep = _host_prep(inputs)
    in_maps = []
    for b in range(B):
        patches = np.asarray(inputs["patches"][b]).astype(np.float32)
        pT = np.ascontiguousarray(patches.T.reshape(PT, P, 256))
        in_maps.append(
            {
                "patchesT": pT,
                "w_img": prep["w_img"],
                "clsstorT": prep["clsstorT"],
                "CC": prep["CC"],
                "SS": prep["SS"],
                "perm": prep["perm"],
                "expsink": prep["expsink"],
                "wq": prep["wq"],
                "wk": prep["wk"],
                "wv": prep["wv"],
                "wo": prep["wo"],
                "rt": prep["rt"],
                "w1": np.ascontiguousarray(prep["w1"][:, b]),
                "w3": np.ascontiguousarray(prep["w3"][:, b]),
                "w2": np.ascontiguousarray(prep["w2"][:, b]),
                "goffs": np.array(
                    [[bb * (D + E) + D + b] for bb in range(B)], np.int32
                ),
            }
        )
    if _NC_CACHE is None:
        _NC_CACHE = build_nc()
    nc = _NC_CACHE
    trace = bool(int(os.environ.get("KERNEL_TRACE", "0")))
    if trace:
        try:
            import axon_profile_hook

            axon_profile_hook.install()
        except Exception:
            pass
    res = run_bass_kernel_spmd(nc, in_maps, core_ids=list(range(B)), trace=trace)
    LAST_RESULT = res
    out = np.stack([res.results[b]["out"] for b in range(B)], 0)
    final_w = np.asarray(inputs["final_norm_w"]).astype(np.float32)
    return out * final_w[None, None, :]
